# revision 70
# baseline (speedup 1.0000x reference)
"""AgentAttention Trainium2 kernel: 8-core data-parallel over batch.

v3: cross-PAIR software pipeline on top of v2's cross-batch fillers.
Iteration i runs scores+exps of head-pair p on PE/scalar while the pv
stages of pair p-1 consume exps computed a full iteration (~17us) earlier,
taking the scalar exp chain off the PE critical path (v2 stalled ~124
times/kernel on exp semaphores). Norm chains are split into pre (copy sumexp
row from psum -> recip -> gpsimd broadcast) and mul phases so the in-order
vector queue never head-of-line blocks on gpsimd. aoT aliases the dead q
tiles of qkT (the tile framework serializes the WAR), freeing 14KB/partition
and letting proj(b) units 2-9 defer into batch b+1 as PE fillers -- the
last batch (which has no next-batch qkv work) rations them across its
iterations. v3.1: bias-table DMAs issued before the big weight loads
(first q-evac stalled 5.8us on bqkp), bf16 output DMA with host upcast
(halves 15.6MB of out traffic; +0.2% quantization, rel err 3.5e-3 ->
4.5e-3 vs the 2e-2 gate), av ones-column memset once per rotation slot.
v3.2: filler-starved iterations (late b3) weave scores between pv2 and
pv3 so the av vector chain hides under score matmuls; wq DMA in q/k/v
column thirds. 605.7us baseline -> 548.0us.

Hard-won negative results (do not redo):
  - gpsimd cannot read PSUM (walrus rejects; custom-DVE recip from psum
    returns garbage on HW even though CoreSim passes).
  - gpsimd tensor_add for the pooling starves the norm broadcasts via
    library swaps: +140us.
  - dma_start on the scalar queue head-of-line blocks exps: +120us.
  - 2-bank psum claims with merged exps/evacs halve scalar ops but the
    4-deep rotation + coarser evac latency cost +56us net.
  - fp8 DoubleRow for qkv/proj GEMMs: e4m3's ~3% RMS error vs the 2e-2
    max-abs/absmax gate (= 3.5e-3 abs diff) fails at near-absmax outputs.
  - matmul PSUM dst at a sub-bank column offset (256B tried) raises a
    runtime exec fault; transposes tolerate sub-bank offsets, regular
    matmuls do not -- dsts must start at a bank boundary.
  - computing s2 directly transposed ([keys, agents] via a block-diagonal
    zero-padded agT) kills the 10 PE transposes + eT copies per pair but
    needs 10 one-bank psum claims and 10 small exps per pair: +83us net
    (claim-rotation stalls inside the scores stage dominate).
  - interleaving the tiny bqkp DMA between the wq half-loads (to exploit
    issue-order-cumulative completion waits) plus front-loading drain proj
    units: +100us -- do not reorder the const-DMA prologue.

Layouts (per core, 4 batches):
  xT      [4, 768, 1176] bf16  (c-major x)
  qkT     c-major q,k: 12 sbuf tiles [128, 1176] (tiles 0-5 = q, 6-11 = k);
          q tiles are overwritten in place by attention output (aoT alias)
  v_ext   pos-major v with per-head ones column (col 64): 10 tiles [128, 12*65]
  agT     pooled agent tokens (sums over 4x4 blocks), c-major [128, 49] x6
  aoT     = qkT[0:6] (bf16) -> proj -> out
Matmuls bf16, fp32 psum (uniform pool of 8 one-bank tiles [128,512]).
Softmax scale folded into ACT exp scale (0.125 stage1; 0.125/16 stages 2/3
-- agent tokens are pooled SUMS). qk bias via per-partition activation bias.
"""

import sys

sys.path.insert(0, "/opt/trn_rl_repo")

import numpy as np
import ml_dtypes

import concourse.bass as bass
import concourse.mybir as mybir
import concourse.tile as tile
from concourse import bacc, bass_utils
from concourse.masks import make_identity

BF = mybir.dt.bfloat16
F32 = mybir.dt.float32
AF = mybir.ActivationFunctionType

N_CORES = 8
B, N, C = 32, 1176, 768
NB = B // N_CORES
H, HD = 12, 64
N_MT, N_S = 392, 784
A = 49
SCALE1 = 0.125
SCALE23 = 0.125 / 16.0

POS_T = [(pt * 128, min(128, N - pt * 128)) for pt in range(10)]
KEY1_T = [(0, 128), (128, 128), (256, 128), (384, 8)]
NCHUNK = [(0, 392), (392, 392), (784, 392)]
CCHUNK = [(0, 512), (512, 256)]
TSP = 116  # transpose chunk col spacing (>=113, even)


def build_program():
    nc = bacc.Bacc("TRN2", debug=False, num_devices=N_CORES)

    xT_d = nc.dram_tensor("xT", [NB, C, N], BF, kind="ExternalInput").ap()
    wqkT_d = nc.dram_tensor("wqkT", [C, 3 * C], BF, kind="ExternalInput").ap()
    wpjT_d = nc.dram_tensor("wpjT", [C, C], BF, kind="ExternalInput").ap()
    vbb_d = nc.dram_tensor("vbb", [128, C], BF, kind="ExternalInput").ap()
    bqkp_d = nc.dram_tensor("bqkp", [128, 12], F32, kind="ExternalInput").ap()
    pbb_d = nc.dram_tensor("pbb", [128, C], BF, kind="ExternalInput").ap()
    # bf16 output (host upcasts): halves 15.6MB of out-DMA traffic and the
    # end-of-kernel drain; adds ~0.2% quantization, well inside the gate
    out_d = nc.dram_tensor("out", [NB, N, C], BF, kind="ExternalOutput").ap()

    with tc_ctx(nc) as (tc, cpool, wpool, hpool, ppool):
        # ---- one-time constants/weights ----
        wq = [
            cpool.tile([128, 3 * C], BF, tag=f"wq{i}", name=f"wq{i}") for i in range(6)
        ]
        wp = [cpool.tile([128, C], BF, tag=f"wp{i}", name=f"wp{i}") for i in range(6)]
        # tiny bias tables FIRST: the first q-evac needs bqkp and stalled
        # 5.8us queued behind the big weight loads
        bqkp = cpool.tile([128, 12], F32, tag="bqkp")
        nc.sync.dma_start(bqkp[:], bqkp_d[:])
        vb_bc = cpool.tile([128, C], BF, tag="vb_bc")
        nc.sync.dma_start(vb_bc[:], vbb_d[:])
        pb_bc = cpool.tile([128, C], BF, tag="pb_bc")
        nc.sync.dma_start(pb_bc[:], pbb_d[:])
        # split wq loads into q/k/v column thirds (same issue order as the
        # proven prologue, finer grain): the first six q_units depend only on
        # the q-column loads, so the opening ldweights waits ~3.6us not ~10us
        for cc in range(3):
            for i in range(6):
                nc.sync.dma_start(
                    wq[i][:, cc * C : (cc + 1) * C],
                    wqkT_d[128 * i : 128 * (i + 1), cc * C : (cc + 1) * C],
                )
        for i in range(6):
            nc.sync.dma_start(wp[i][:], wpjT_d[128 * i : 128 * (i + 1), :])
        ident = cpool.tile([128, 128], BF, tag="ident")
        make_identity(nc, ident[:])

        pv2_calls = [0]

        # per-batch tile handles (rotated via tags, bufs=2)
        xT = {}
        qkT = {}
        v_ext = {}
        agT = {}
        aoT = {}

        def psum(name):
            return ppool.tile([128, 512], F32, tag="P", name=name, bufs=8)

        def load_x(b):
            xT[b] = [
                hpool.tile([128, N], BF, tag=f"xT{i}", name=f"xT{i}", bufs=2)
                for i in range(6)
            ]
            eng = nc.scalar if b == 0 else nc.sync
            for i in range(6):
                eng.dma_start(xT[b][i][:], xT_d[b, 128 * i : 128 * (i + 1), :])

        def q_unit(b, m):
            # qkT[m] c-major [128, 1176] for q (m<6) / k (m>=6) rows
            if m == 0:
                qkT[b] = [None] * 12
            t = hpool.tile([128, N], BF, tag=f"qkT{m}", name=f"qkT{m}", bufs=2)
            qkT[b][m] = t
            for n0, nsz in NCHUNK:
                ps = psum("psQ")
                for kt in range(6):
                    nc.tensor.matmul(
                        ps[:, 0:nsz],
                        wq[kt][:, 128 * m : 128 * (m + 1)],
                        xT[b][kt][:, n0 : n0 + nsz],
                        start=(kt == 0),
                        stop=(kt == 5),
                    )
                # evac on scalar engine (gpsimd cannot read PSUM)
                nc.scalar.activation(
                    t[:, n0 : n0 + nsz],
                    ps[:, 0:nsz],
                    AF.Identity,
                    bias=bqkp[:, m : m + 1],
                )

        def v_unit(b, pt):
            # pos-major v_ext [psz, 12*65] with ones col at 64 of each head.
            # both c-chunks in one 2-bank claim -> ONE merged evac add
            p0, psz = POS_T[pt]
            if pt == 0:
                v_ext[b] = [None] * 10
            vt = hpool.tile([128, H * 65], BF, tag=f"vx{pt}", name=f"vx{pt}", bufs=2)
            v_ext[b][pt] = vt
            if b < 2:
                # two rotation slots; evac only writes the 64 v columns, so
                # ones persist across later batches
                nc.vector.memset(
                    vt[:].rearrange("p (h e) -> p h e", e=65)[:, :, 64:65], 1.0
                )
            for ci, (c0, csz) in enumerate(CCHUNK):
                ps = psum("psV")
                for kt in range(6):
                    nc.tensor.matmul(
                        ps[0:psz, 0:csz],
                        xT[b][kt][:, p0 : p0 + psz],
                        wq[kt][:, 2 * C + c0 : 2 * C + c0 + csz],
                        start=(kt == 0),
                        stop=(kt == 5),
                    )
                nh = csz // 64
                h0 = c0 // 64
                nc.vector.tensor_add(
                    vt[0:psz].rearrange("p (h e) -> p h e", e=65)[
                        :, h0 : h0 + nh, 0:64
                    ],
                    ps[0:psz, 0:csz].rearrange("p (h d) -> p h d", d=64),
                    vb_bc[0:psz, c0 : c0 + csz].rearrange("p (h d) -> p h d", d=64),
                )

        def pool_ct(b, ct):
            # sum 4x4 blocks of q_s -> agT (c-major). On VECTOR: gpsimd must
            # stay broadcast-only (lib swaps + in-order blocking starve the
            # norm-chain broadcasts otherwise)
            if ct == 0:
                agT[b] = []
            t1 = wpool.tile([128, 196], F32, tag="t1", bufs=1)
            qs = qkT[b][ct][:, N_MT:N]  # [128, 784], idx = i*28 + aj*4 + dj
            q4 = qs.rearrange("p (x dj) -> p x dj", dj=4)
            nc.vector.tensor_add(t1[:, 0:196], q4[:, :, 0:1], q4[:, :, 1:2])
            nc.vector.tensor_add(t1[:, 0:196], t1[:, 0:196], q4[:, :, 2:3])
            nc.vector.tensor_add(t1[:, 0:196], t1[:, 0:196], q4[:, :, 3:4])
            ag = hpool.tile([128, A], BF, tag=f"ag{ct}", name=f"ag{ct}", bufs=2)
            agT[b].append(ag)
            # t1 idx = 28*ai + 7*di + aj -> view (ai, aj, di)
            t4 = t1[:, 0:196].rearrange("p (ai di aj) -> p ai aj di", ai=7, di=4)
            t2 = wpool.tile([128, A], F32, tag="t2")
            nc.vector.tensor_add(t2[:, 0:A], t4[:, :, :, 0:1], t4[:, :, :, 1:2])
            nc.vector.tensor_add(t2[:, 0:A], t2[:, 0:A], t4[:, :, :, 2:3])
            nc.vector.tensor_add(ag[:, 0:A], t2[:, 0:A], t4[:, :, :, 3:4])

        def norm_pre(pv, c0):
            # recip of the psum sumexp row, broadcast to 64 partitions.
            # (custom-DVE recip reading PSUM directly returns garbage on HW;
            # stage the sumexp row through SBUF first.) Split from the mul so
            # the in-order vector queue never head-of-line blocks on gpsimd.
            se = wpool.tile([1, 392], F32, tag="se", bufs=2)
            nc.vector.tensor_copy(se[:, 0:392], pv[64:65, c0 : c0 + 392])
            rc = wpool.tile([1, 392], F32, tag="rc", bufs=2)
            nc.vector.reciprocal_approx_fast(out=rc[:, 0:392], in_=se[:, 0:392])
            bc = wpool.tile([64, 392], F32, tag="bc", bufs=6)
            nc.gpsimd.partition_broadcast(bc[:], rc[0:1, 0:392])
            return bc

        def norm_mul(pv, c0, bc, dst):
            nc.vector.tensor_mul(dst, pv[0:64, c0 : c0 + 392], bc[:])

        # ---- attention for one head pair, split into schedulable chunks ----
        def pair_scores_s1(b, p2, st):
            qt = p2
            # stage 1 scores first: [keys, queries] per head over 4 key chunks.
            # Claim order matches exp (= psum evacuation) order so the 8-bank
            # rotation never waits, and e1 (pv1's dep) is computed earliest.
            st["s1"] = []
            for hp in range(2):
                qo = 64 * hp
                chunks = []
                st["s1"].append(chunks)
                for k0, ksz in KEY1_T:
                    ps = psum("psS1")
                    chunks.append(ps)
                    nc.tensor.matmul(
                        ps[0:ksz, 0:392],
                        qkT[b][6 + qt][qo : qo + 64, k0 : k0 + ksz],
                        qkT[b][qt][qo : qo + 64, 0:N_MT],
                        start=True,
                        stop=True,
                    )
            # e1 exps issue right after the s1 matmuls so pv1 of the NEXT
            # iteration never waits
            st["e1"] = []
            for hp in range(2):
                e1s = []
                st["e1"].append(e1s)
                for j, (k0, ksz) in enumerate(KEY1_T):
                    e1 = wpool.tile([128, 392], BF, tag="e1", name="e1", bufs=16)
                    e1s.append(e1)
                    nc.scalar.activation(
                        e1[0:ksz, 0:392],
                        st["s1"][hp][j][0:ksz, 0:392],
                        AF.Exp,
                        scale=SCALE1,
                    )

        def pair_scores_s23(b, p2, st):
            qt = p2
            # stage 2 scores: [49x2 packed, keys] over 3 chunks
            st["s2"] = []
            for n0, nsz in NCHUNK:
                ps = psum("psS2")
                st["s2"].append(ps)
                for hp in range(2):
                    qo = 64 * hp
                    nc.tensor.matmul(
                        ps[qo : qo + 49, 0:nsz],
                        agT[b][qt][qo : qo + 64, 0:A],
                        qkT[b][6 + qt][qo : qo + 64, n0 : n0 + nsz],
                        start=True,
                        stop=True,
                    )
            # stage 3 scores: [49x2 packed (agents), queries] over 2 chunks
            st["s3"] = []
            for cc in range(2):
                ps = psum("psS3")
                st["s3"].append(ps)
                for hp in range(2):
                    qo = 64 * hp
                    nc.tensor.matmul(
                        ps[qo : qo + 49, 0:392],
                        agT[b][qt][qo : qo + 64, 0:A],
                        qkT[b][qt][qo : qo + 64, N_MT + 392 * cc : N_MT + 392 * (cc + 1)],
                        start=True,
                        stop=True,
                    )
            e2 = wpool.tile([128, N], BF, tag="e2", bufs=2)
            st["e2"] = e2
            for j, (n0, nsz) in enumerate(NCHUNK):
                nc.scalar.activation(
                    e2[0:113, n0 : n0 + nsz],
                    st["s2"][j][0:113, 0:nsz],
                    AF.Exp,
                    scale=SCALE23,
                )
            st["e3"] = []
            for cc in range(2):
                e3 = wpool.tile([128, 392], BF, tag="e3", name="e3", bufs=4)
                st["e3"].append(e3)
                nc.scalar.activation(
                    e3[0:113, 0:392], st["s3"][cc][0:113, 0:392], AF.Exp, scale=SCALE23
                )

        def pair_scores(b, p2, st):
            pair_scores_s1(b, p2, st)
            pair_scores_s23(b, p2, st)

        def pair_pv1_mm(b, p2, st):
            st["pv1"] = []
            for hp in range(2):
                pv = psum("psPV1")
                st["pv1"].append(pv)
                for j, (k0, ksz) in enumerate(KEY1_T):
                    nc.tensor.matmul(
                        pv[0:65, 0:392],
                        v_ext[b][j][0:ksz, 65 * (2 * p2 + hp) : 65 * (2 * p2 + hp) + 65],
                        st["e1"][hp][j][0:ksz, 0:392],
                        start=(j == 0),
                        stop=(j == 3),
                    )

        def pair_pv1_norm_pre(b, p2, st):
            st["bc1"] = [norm_pre(st["pv1"][hp], 0) for hp in range(2)]

        def pair_pv1_norm_mul(b, p2, st):
            qt = p2
            for hp in range(2):
                qo = 64 * hp
                norm_mul(
                    st["pv1"][hp], 0, st["bc1"][hp],
                    aoT[b][qt][qo : qo + 64, 0:N_MT],
                )

        def pair_transp(b, p2, st):
            # [113, keys] -> [keys, 113] in 10 chunks, via identity matmul
            st["eT"] = []
            for half in range(2):
                trp = ppool.tile([128, 5 * TSP], BF, tag="P", name="psTr", bufs=8)
                for kk in range(5):
                    kt = 5 * half + kk
                    p0, psz = POS_T[kt]
                    nc.tensor.transpose(
                        trp[0:psz, TSP * kk : TSP * kk + 113],
                        st["e2"][0:113, p0 : p0 + psz],
                        ident[0:113, 0:113],
                    )
                eT = wpool.tile([128, 5 * TSP], BF, tag="e2T", bufs=4)
                st["eT"].append(eT)
                # evac on vector: scalar is exp-saturated in the pair slot
                nc.vector.tensor_copy(eT[:, 0 : 5 * TSP], trp[:, 0 : 5 * TSP])

        def pair_pv2(b, p2, st, do_memset=False):
            # both heads per matmul: lhsT = full transposed tile (garbage rows
            # 49:63 only pollute unused output rows), rhs = 129-wide v_ext
            # slice [v_h0 | ones | v_h1]; the ones col yields both heads'
            # sumexp at out col 64. 10 matmuls instead of 20.
            do_memset = do_memset or pv2_calls[0] < 2
            pv2_calls[0] += 1
            pv2 = psum("psPV2")
            for kt, (p0, psz) in enumerate(POS_T):
                eT = st["eT"][kt // 5]
                cof = TSP * (kt % 5)
                nc.tensor.matmul(
                    pv2[0:113, 0:129],
                    eT[0:psz, cof : cof + 113],
                    v_ext[b][kt][0:psz, 130 * p2 : 130 * p2 + 129],
                    start=(kt == 0),
                    stop=(kt == 9),
                )
            av = wpool.tile([128, 65], BF, tag="avx", bufs=2)
            st["av"] = av
            if do_memset:
                # two rotation slots; ts_muls only write cols 0:64, so the
                # ones column persists across later pairs
                nc.vector.memset(av[0:113, 64:65], 1.0)
            avr = wpool.tile([128, 1], F32, tag="avr", bufs=2)
            nc.vector.reciprocal(avr[0:113, 0:1], pv2[0:113, 64:65])
            nc.vector.tensor_scalar_mul(av[0:49, 0:64], pv2[0:49, 0:64], avr[0:49, 0:1])
            nc.vector.tensor_scalar_mul(
                av[64:113, 0:64], pv2[64:113, 65:129], avr[64:113, 0:1]
            )

        def pair_pv3_mm(b, p2, st):
            st["pv3"] = []
            for hp in range(2):
                for cc in range(2):
                    pv = psum("psPV3")
                    st["pv3"].append(pv)
                    nc.tensor.matmul(
                        pv[0:65, 0:392],
                        st["av"][64 * hp : 64 * hp + 49, 0:65],
                        st["e3"][cc][64 * hp : 64 * hp + 49, 0:392],
                        start=True,
                        stop=True,
                    )

        def pair_pv3_norm_pre(b, p2, st):
            st["bc3"] = [norm_pre(pv, 0) for pv in st["pv3"]]

        def pair_pv3_norm_mul(b, p2, st):
            qt = p2
            for i, pv in enumerate(st["pv3"]):
                hp, cc = divmod(i, 2)
                norm_mul(
                    pv,
                    0,
                    st["bc3"][i],
                    aoT[b][qt][64 * hp : 64 * hp + 64, N_MT + 392 * cc : N_MT + 392 * (cc + 1)],
                )

        def proj_unit(b, pt):
            p0, psz = POS_T[pt]
            ob = wpool.tile([128, C], BF, tag="osb")
            for c0, csz in CCHUNK:
                ps = psum("psPJ")
                for kt in range(6):
                    nc.tensor.matmul(
                        ps[0:psz, 0:csz],
                        aoT[b][kt][:, p0 : p0 + psz],
                        wp[kt][:, c0 : c0 + csz],
                        start=(kt == 0),
                        stop=(kt == 5),
                    )
                nc.vector.tensor_add(
                    ob[0:psz, c0 : c0 + csz], ps[0:psz, 0:csz], pb_bc[0:psz, c0 : c0 + csz]
                )
            nc.sync.dma_start(out_d[b, p0 : p0 + psz, :], ob[0:psz, :])

        def qk_pool_unit(b, m):
            q_unit(b, m)
            if m < 6:
                pool_ct(b, m)

        def qkv_units(b):
            units = []
            for m in range(12):
                units.append(lambda m=m: qk_pool_unit(b, m))
            for pt in range(10):
                units.append(lambda pt=pt: v_unit(b, pt))
            return units

        # ---- schedule ----
        # aoT[b] aliases qkT[b] q-tiles 0-5: by the time a pair's norms write
        # a tile, that pair's s1/s3 score matmuls (its only q readers) are
        # done -- the tile framework serializes the WAR. Saves 14KB/partition
        # and lets proj(b) units 2-9 defer into batch b+1's pair loop as PE
        # fillers (the last batch finally gets filler work).
        load_x(0)
        load_x(1)
        for u in qkv_units(0):
            u()

        prev = None
        for b in range(NB):
            aoT[b] = qkT[b][0:6]
            # fill order matters: proj(b-1) reads qkT[b-1] (same buf parity
            # as qkv(b+1)'s q_unit writes), so proj units must drain first.
            fill = []
            if b > 0:
                fill += [lambda pt=pt, pb=b - 1: proj_unit(pb, pt) for pt in range(2, 10)]
            if b + 1 < NB:
                fill += list(qkv_units(b + 1))
            if b + 2 < NB:
                load_x(b + 2)
            fi = 0
            # last batch has few fillers (proj of b-1 only): ration them
            # across iterations instead of exhausting them in the first two
            ration = 2 if b == NB - 1 else 10**9
            it_taken = [0]

            def take(n):
                nonlocal fi
                for _ in range(n):
                    if fi < len(fill) and it_taken[0] < ration:
                        fill[fi]()
                        fi += 1
                        it_taken[0] += 1

            # cross-pair software pipeline: iteration runs scores(+exps) of
            # pair p while the pv-stages of pair p-1 consume exps computed a
            # full iteration (~17us) earlier -- the scalar exp chain leaves
            # the PE critical path entirely.
            for p2 in range(6):
                st = {}
                it_taken[0] = 0
                if prev is None:
                    pair_scores(b, p2, st)
                    take(2)
                elif prev[1] == 5:
                    # batch boundary: pv-stage of (b-1, 5) woven with
                    # scores(b, 0). No takes until its pv3 norms land --
                    # the proj(b-1) fillers read the s-region they write
                    # (taking one earlier would deadlock the PE queue).
                    pb_, pp_, pst = prev
                    pair_pv1_mm(pb_, pp_, pst)
                    pair_transp(pb_, pp_, pst)
                    pair_pv1_norm_pre(pb_, pp_, pst)
                    pair_scores(b, p2, st)
                    pair_pv2(pb_, pp_, pst)
                    pair_pv1_norm_mul(pb_, pp_, pst)
                    proj_unit(pb_, 0)
                    pair_pv3_mm(pb_, pp_, pst)
                    pair_pv3_norm_pre(pb_, pp_, pst)
                    pair_pv3_norm_mul(pb_, pp_, pst)
                    proj_unit(pb_, 1)
                    take(4)
                elif fi < len(fill):
                    pb_, pp_, pst = prev
                    pair_scores_s1(b, p2, st)
                    pair_pv1_mm(pb_, pp_, pst)
                    pair_scores_s23(b, p2, st)
                    pair_transp(pb_, pp_, pst)
                    pair_pv1_norm_pre(pb_, pp_, pst)
                    take(1)
                    pair_pv2(pb_, pp_, pst)
                    pair_pv1_norm_mul(pb_, pp_, pst)
                    take(1)
                    pair_pv3_mm(pb_, pp_, pst)
                    pair_pv3_norm_pre(pb_, pp_, pst)
                    take(1)
                    pair_pv3_norm_mul(pb_, pp_, pst)
                    take(2)
                else:
                    # filler-starved (late b3): weave scores between pv2 and
                    # pv3 so the av vector chain hides under the score
                    # matmuls (same shape as the batch-boundary branch)
                    pb_, pp_, pst = prev
                    pair_pv1_mm(pb_, pp_, pst)
                    pair_transp(pb_, pp_, pst)
                    pair_pv1_norm_pre(pb_, pp_, pst)
                    pair_pv2(pb_, pp_, pst)
                    pair_pv1_norm_mul(pb_, pp_, pst)
                    pair_scores(b, p2, st)
                    pair_pv3_mm(pb_, pp_, pst)
                    pair_pv3_norm_pre(pb_, pp_, pst)
                    pair_pv3_norm_mul(pb_, pp_, pst)
                prev = (b, p2, st)
            it_taken[0] = -(10**9)  # drain remaining fillers unrationed
            take(len(fill))

        # drain: pv-stage of the final pair + last batch's proj
        pb_, pp_, pst = prev
        pair_pv1_mm(pb_, pp_, pst)
        pair_transp(pb_, pp_, pst)
        pair_pv1_norm_pre(pb_, pp_, pst)
        pair_pv2(pb_, pp_, pst)
        pair_pv1_norm_mul(pb_, pp_, pst)
        proj_unit(pb_, 0)
        pair_pv3_mm(pb_, pp_, pst)
        pair_pv3_norm_pre(pb_, pp_, pst)
        pair_pv3_norm_mul(pb_, pp_, pst)
        proj_unit(pb_, 1)
        for pt in range(2, 10):
            proj_unit(pb_, pt)

    nc.compile()
    return nc


def tc_ctx(nc):
    from contextlib import contextmanager

    @contextmanager
    def ctx():
        with tile.TileContext(nc) as tc, nc.allow_low_precision(reason="attn bf16"):
            with (
                tc.tile_pool(name="const", bufs=1) as cpool,
                tc.tile_pool(name="work", bufs=2) as wpool,
                tc.tile_pool(name="hold", bufs=1) as hpool,
                tc.tile_pool(name="psum", bufs=8, space="PSUM") as ppool,
            ):
                yield tc, cpool, wpool, hpool, ppool

    return ctx()


_PROGRAM = None


def _get_program():
    global _PROGRAM
    if _PROGRAM is None:
        _PROGRAM = build_program()
    return _PROGRAM


def _prep_inputs(x, qkv_w, qkv_b, proj_w, proj_b):
    bf = ml_dtypes.bfloat16
    x = np.asarray(x, dtype=np.float32)
    xT = np.ascontiguousarray(x.transpose(0, 2, 1)).astype(bf)  # [B, C, N]
    wqkT = np.ascontiguousarray(np.asarray(qkv_w, dtype=np.float32).T).astype(bf)
    wpjT = np.ascontiguousarray(np.asarray(proj_w, dtype=np.float32).T).astype(bf)
    qb = np.asarray(qkv_b, dtype=np.float32)
    vbb = np.broadcast_to(qb[2 * 768 :].astype(bf), (128, 768)).copy()
    bqkp = np.ascontiguousarray(qb[: 2 * 768].reshape(12, 128).T).astype(np.float32)
    pbb = np.broadcast_to(
        np.asarray(proj_b, dtype=np.float32).astype(bf), (128, 768)
    ).copy()
    in_maps = []
    for c in range(N_CORES):
        in_maps.append(
            {
                "xT": np.ascontiguousarray(xT[c * NB : (c + 1) * NB]),
                "wqkT": wqkT,
                "wpjT": wpjT,
                "vbb": vbb,
                "bqkp": bqkp,
                "pbb": pbb,
            }
        )
    return in_maps


def kernel(x, qkv_w, qkv_b, proj_w, proj_b, t_h=14, t_w=14, s_h=28, s_w=28, **kw):
    nc = _get_program()
    in_maps = _prep_inputs(x, qkv_w, qkv_b, proj_w, proj_b)
    res = bass_utils.run_bass_kernel_spmd(nc, in_maps, core_ids=list(range(N_CORES)))
    out = np.concatenate([res.results[c]["out"] for c in range(N_CORES)], axis=0)
    return np.asarray(out, dtype=np.float32)


if __name__ == "__main__":
    build_program()
    print("program built OK")



# revision 71
# speedup vs baseline: 1.0018x; 1.0018x over previous
"""AgentAttention Trainium2 kernel: 8-core data-parallel over batch.

v3: cross-PAIR software pipeline on top of v2's cross-batch fillers.
Iteration i runs scores+exps of head-pair p on PE/scalar while the pv
stages of pair p-1 consume exps computed a full iteration (~17us) earlier,
taking the scalar exp chain off the PE critical path (v2 stalled ~124
times/kernel on exp semaphores). Norm chains are split into pre (copy sumexp
row from psum -> recip -> gpsimd broadcast) and mul phases so the in-order
vector queue never head-of-line blocks on gpsimd. aoT aliases the dead q
tiles of qkT (the tile framework serializes the WAR), freeing 14KB/partition
and letting proj(b) units 2-9 defer into batch b+1 as PE fillers -- the
last batch (which has no next-batch qkv work) rations them across its
iterations. v3.1: bias-table DMAs issued before the big weight loads
(first q-evac stalled 5.8us on bqkp), bf16 output DMA with host upcast
(halves 15.6MB of out traffic; +0.2% quantization, rel err 3.5e-3 ->
4.5e-3 vs the 2e-2 gate), av ones-column memset once per rotation slot.
v3.2: filler-starved iterations (late b3) weave scores between pv2 and
pv3 so the av vector chain hides under score matmuls; wq DMA in q/k/v
column thirds. 605.7us baseline -> 548.0us.

Hard-won negative results (do not redo):
  - gpsimd cannot read PSUM (walrus rejects; custom-DVE recip from psum
    returns garbage on HW even though CoreSim passes).
  - gpsimd tensor_add for the pooling starves the norm broadcasts via
    library swaps: +140us.
  - dma_start on the scalar queue head-of-line blocks exps: +120us.
  - 2-bank psum claims with merged exps/evacs halve scalar ops but the
    4-deep rotation + coarser evac latency cost +56us net.
  - fp8 DoubleRow for qkv/proj GEMMs: e4m3's ~3% RMS error vs the 2e-2
    max-abs/absmax gate (= 3.5e-3 abs diff) fails at near-absmax outputs.
  - matmul PSUM dst at a sub-bank column offset (256B tried) raises a
    runtime exec fault; transposes tolerate sub-bank offsets, regular
    matmuls do not -- dsts must start at a bank boundary.
  - computing s2 directly transposed ([keys, agents] via a block-diagonal
    zero-padded agT) kills the 10 PE transposes + eT copies per pair but
    needs 10 one-bank psum claims and 10 small exps per pair: +83us net
    (claim-rotation stalls inside the scores stage dominate).
  - interleaving the tiny bqkp DMA between the wq half-loads (to exploit
    issue-order-cumulative completion waits) plus front-loading drain proj
    units: +100us -- do not reorder the const-DMA prologue.

Layouts (per core, 4 batches):
  xT      [4, 768, 1176] bf16  (c-major x)
  qkT     c-major q,k: 12 sbuf tiles [128, 1176] (tiles 0-5 = q, 6-11 = k);
          q tiles are overwritten in place by attention output (aoT alias)
  v_ext   pos-major v with per-head ones column (col 64): 10 tiles [128, 12*65]
  agT     pooled agent tokens (sums over 4x4 blocks), c-major [128, 49] x6
  aoT     = qkT[0:6] (bf16) -> proj -> out
Matmuls bf16, fp32 psum (uniform pool of 8 one-bank tiles [128,512]).
Softmax scale folded into ACT exp scale (0.125 stage1; 0.125/16 stages 2/3
-- agent tokens are pooled SUMS). qk bias via per-partition activation bias.
"""

import sys

sys.path.insert(0, "/opt/trn_rl_repo")

import numpy as np
import ml_dtypes

import concourse.bass as bass
import concourse.mybir as mybir
import concourse.tile as tile
from concourse import bacc, bass_utils
from concourse.masks import make_identity

BF = mybir.dt.bfloat16
F32 = mybir.dt.float32
AF = mybir.ActivationFunctionType

N_CORES = 8
B, N, C = 32, 1176, 768
NB = B // N_CORES
H, HD = 12, 64
N_MT, N_S = 392, 784
A = 49
SCALE1 = 0.125
SCALE23 = 0.125 / 16.0

POS_T = [(pt * 128, min(128, N - pt * 128)) for pt in range(10)]
KEY1_T = [(0, 128), (128, 128), (256, 128), (384, 8)]
NCHUNK = [(0, 392), (392, 392), (784, 392)]
CCHUNK = [(0, 512), (512, 256)]
TSP = 116  # transpose chunk col spacing (>=113, even)


def build_program():
    nc = bacc.Bacc("TRN2", debug=False, num_devices=N_CORES)

    xT_d = nc.dram_tensor("xT", [NB, C, N], BF, kind="ExternalInput").ap()
    wqkT_d = nc.dram_tensor("wqkT", [C, 3 * C], BF, kind="ExternalInput").ap()
    wpjT_d = nc.dram_tensor("wpjT", [C, C], BF, kind="ExternalInput").ap()
    vbb_d = nc.dram_tensor("vbb", [128, C], BF, kind="ExternalInput").ap()
    bqkp_d = nc.dram_tensor("bqkp", [128, 12], F32, kind="ExternalInput").ap()
    pbb_d = nc.dram_tensor("pbb", [128, C], BF, kind="ExternalInput").ap()
    # bf16 output (host upcasts): halves 15.6MB of out-DMA traffic and the
    # end-of-kernel drain; adds ~0.2% quantization, well inside the gate
    out_d = nc.dram_tensor("out", [NB, N, C], BF, kind="ExternalOutput").ap()

    with tc_ctx(nc) as (tc, cpool, wpool, hpool, ppool):
        # ---- one-time constants/weights ----
        wq = [
            cpool.tile([128, 3 * C], BF, tag=f"wq{i}", name=f"wq{i}") for i in range(6)
        ]
        wp = [cpool.tile([128, C], BF, tag=f"wp{i}", name=f"wp{i}") for i in range(6)]
        # tiny bias tables FIRST: the first q-evac needs bqkp and stalled
        # 5.8us queued behind the big weight loads
        bqkp = cpool.tile([128, 12], F32, tag="bqkp")
        nc.sync.dma_start(bqkp[:], bqkp_d[:])
        vb_bc = cpool.tile([128, C], BF, tag="vb_bc")
        nc.sync.dma_start(vb_bc[:], vbb_d[:])
        pb_bc = cpool.tile([128, C], BF, tag="pb_bc")
        nc.sync.dma_start(pb_bc[:], pbb_d[:])
        # split wq loads into q/k/v column thirds (same issue order as the
        # proven prologue, finer grain): the first six q_units depend only on
        # the q-column loads, so the opening ldweights waits ~3.6us not ~10us
        for cc in range(3):
            for i in range(6):
                nc.sync.dma_start(
                    wq[i][:, cc * C : (cc + 1) * C],
                    wqkT_d[128 * i : 128 * (i + 1), cc * C : (cc + 1) * C],
                )
        for i in range(6):
            nc.sync.dma_start(wp[i][:], wpjT_d[128 * i : 128 * (i + 1), :])
        ident = cpool.tile([128, 128], BF, tag="ident")
        make_identity(nc, ident[:])

        pv2_calls = [0]

        # per-batch tile handles (rotated via tags, bufs=2)
        xT = {}
        qkT = {}
        v_ext = {}
        agT = {}
        aoT = {}

        def psum(name):
            return ppool.tile([128, 512], F32, tag="P", name=name, bufs=8)

        def load_x(b):
            xT[b] = [
                hpool.tile([128, N], BF, tag=f"xT{i}", name=f"xT{i}", bufs=2)
                for i in range(6)
            ]
            eng = nc.scalar if b == 0 else nc.sync
            for i in range(6):
                eng.dma_start(xT[b][i][:], xT_d[b, 128 * i : 128 * (i + 1), :])

        def q_unit(b, m):
            # qkT[m] c-major [128, 1176] for q (m<6) / k (m>=6) rows
            if m == 0:
                qkT[b] = [None] * 12
            t = hpool.tile([128, N], BF, tag=f"qkT{m}", name=f"qkT{m}", bufs=2)
            qkT[b][m] = t
            for n0, nsz in NCHUNK:
                ps = psum("psQ")
                for kt in range(6):
                    nc.tensor.matmul(
                        ps[:, 0:nsz],
                        wq[kt][:, 128 * m : 128 * (m + 1)],
                        xT[b][kt][:, n0 : n0 + nsz],
                        start=(kt == 0),
                        stop=(kt == 5),
                    )
                # evac on scalar engine (gpsimd cannot read PSUM)
                nc.scalar.activation(
                    t[:, n0 : n0 + nsz],
                    ps[:, 0:nsz],
                    AF.Identity,
                    bias=bqkp[:, m : m + 1],
                )

        def v_unit(b, pt):
            # pos-major v_ext [psz, 12*65] with ones col at 64 of each head.
            # both c-chunks in one 2-bank claim -> ONE merged evac add
            p0, psz = POS_T[pt]
            if pt == 0:
                v_ext[b] = [None] * 10
            vt = hpool.tile([128, H * 65], BF, tag=f"vx{pt}", name=f"vx{pt}", bufs=2)
            v_ext[b][pt] = vt
            if b < 2:
                # two rotation slots; evac only writes the 64 v columns, so
                # ones persist across later batches
                nc.vector.memset(
                    vt[:].rearrange("p (h e) -> p h e", e=65)[:, :, 64:65], 1.0
                )
            for ci, (c0, csz) in enumerate(CCHUNK):
                ps = psum("psV")
                for kt in range(6):
                    nc.tensor.matmul(
                        ps[0:psz, 0:csz],
                        xT[b][kt][:, p0 : p0 + psz],
                        wq[kt][:, 2 * C + c0 : 2 * C + c0 + csz],
                        start=(kt == 0),
                        stop=(kt == 5),
                    )
                nh = csz // 64
                h0 = c0 // 64
                nc.vector.tensor_add(
                    vt[0:psz].rearrange("p (h e) -> p h e", e=65)[
                        :, h0 : h0 + nh, 0:64
                    ],
                    ps[0:psz, 0:csz].rearrange("p (h d) -> p h d", d=64),
                    vb_bc[0:psz, c0 : c0 + csz].rearrange("p (h d) -> p h d", d=64),
                )

        def pool_ct(b, ct):
            # sum 4x4 blocks of q_s -> agT (c-major). On VECTOR: gpsimd must
            # stay broadcast-only (lib swaps + in-order blocking starve the
            # norm-chain broadcasts otherwise)
            if ct == 0:
                agT[b] = []
            t1 = wpool.tile([128, 196], F32, tag="t1", bufs=1)
            qs = qkT[b][ct][:, N_MT:N]  # [128, 784], idx = i*28 + aj*4 + dj
            q4 = qs.rearrange("p (x dj) -> p x dj", dj=4)
            nc.vector.tensor_add(t1[:, 0:196], q4[:, :, 0:1], q4[:, :, 1:2])
            nc.vector.tensor_add(t1[:, 0:196], t1[:, 0:196], q4[:, :, 2:3])
            nc.vector.tensor_add(t1[:, 0:196], t1[:, 0:196], q4[:, :, 3:4])
            ag = hpool.tile([128, A], BF, tag=f"ag{ct}", name=f"ag{ct}", bufs=2)
            agT[b].append(ag)
            # t1 idx = 28*ai + 7*di + aj -> view (ai, aj, di)
            t4 = t1[:, 0:196].rearrange("p (ai di aj) -> p ai aj di", ai=7, di=4)
            t2 = wpool.tile([128, A], F32, tag="t2")
            nc.vector.tensor_add(t2[:, 0:A], t4[:, :, :, 0:1], t4[:, :, :, 1:2])
            nc.vector.tensor_add(t2[:, 0:A], t2[:, 0:A], t4[:, :, :, 2:3])
            nc.vector.tensor_add(ag[:, 0:A], t2[:, 0:A], t4[:, :, :, 3:4])

        def norm_pre(pv, c0):
            # recip of the psum sumexp row, broadcast to 64 partitions.
            # (custom-DVE recip reading PSUM directly returns garbage on HW;
            # stage the sumexp row through SBUF first.) Split from the mul so
            # the in-order vector queue never head-of-line blocks on gpsimd.
            se = wpool.tile([1, 392], F32, tag="se", bufs=2)
            nc.vector.tensor_copy(se[:, 0:392], pv[64:65, c0 : c0 + 392])
            rc = wpool.tile([1, 392], F32, tag="rc", bufs=2)
            nc.vector.reciprocal_approx_fast(out=rc[:, 0:392], in_=se[:, 0:392])
            bc = wpool.tile([64, 392], F32, tag="bc", bufs=6)
            nc.gpsimd.partition_broadcast(bc[:], rc[0:1, 0:392])
            return bc

        def norm_mul(pv, c0, bc, dst):
            nc.vector.tensor_mul(dst, pv[0:64, c0 : c0 + 392], bc[:])

        # ---- attention for one head pair, split into schedulable chunks ----
        def pair_scores_s1(b, p2, st):
            qt = p2
            # stage 1 scores first: [keys, queries] per head over 4 key chunks.
            # Claim order matches exp (= psum evacuation) order so the 8-bank
            # rotation never waits, and e1 (pv1's dep) is computed earliest.
            st["s1"] = []
            for hp in range(2):
                qo = 64 * hp
                chunks = []
                st["s1"].append(chunks)
                for k0, ksz in KEY1_T:
                    ps = psum("psS1")
                    chunks.append(ps)
                    nc.tensor.matmul(
                        ps[0:ksz, 0:392],
                        qkT[b][6 + qt][qo : qo + 64, k0 : k0 + ksz],
                        qkT[b][qt][qo : qo + 64, 0:N_MT],
                        start=True,
                        stop=True,
                    )
            # e1 exps issue right after the s1 matmuls so pv1 of the NEXT
            # iteration never waits
            st["e1"] = []
            for hp in range(2):
                e1s = []
                st["e1"].append(e1s)
                for j, (k0, ksz) in enumerate(KEY1_T):
                    e1 = wpool.tile([128, 392], BF, tag="e1", name="e1", bufs=16)
                    e1s.append(e1)
                    nc.scalar.activation(
                        e1[0:ksz, 0:392],
                        st["s1"][hp][j][0:ksz, 0:392],
                        AF.Exp,
                        scale=SCALE1,
                    )

        def pair_scores_s23(b, p2, st):
            qt = p2
            # stage 2 scores: [49x2 packed, keys] over 3 chunks
            st["s2"] = []
            for n0, nsz in NCHUNK:
                ps = psum("psS2")
                st["s2"].append(ps)
                for hp in range(2):
                    qo = 64 * hp
                    nc.tensor.matmul(
                        ps[qo : qo + 49, 0:nsz],
                        agT[b][qt][qo : qo + 64, 0:A],
                        qkT[b][6 + qt][qo : qo + 64, n0 : n0 + nsz],
                        start=True,
                        stop=True,
                    )
            # stage 3 scores: [49x2 packed (agents), queries] over 2 chunks
            st["s3"] = []
            for cc in range(2):
                ps = psum("psS3")
                st["s3"].append(ps)
                for hp in range(2):
                    qo = 64 * hp
                    nc.tensor.matmul(
                        ps[qo : qo + 49, 0:392],
                        agT[b][qt][qo : qo + 64, 0:A],
                        qkT[b][qt][qo : qo + 64, N_MT + 392 * cc : N_MT + 392 * (cc + 1)],
                        start=True,
                        stop=True,
                    )
            e2 = wpool.tile([128, N], BF, tag="e2", bufs=2)
            st["e2"] = e2
            for j, (n0, nsz) in enumerate(NCHUNK):
                nc.scalar.activation(
                    e2[0:113, n0 : n0 + nsz],
                    st["s2"][j][0:113, 0:nsz],
                    AF.Exp,
                    scale=SCALE23,
                )
            st["e3"] = []
            for cc in range(2):
                e3 = wpool.tile([128, 392], BF, tag="e3", name="e3", bufs=4)
                st["e3"].append(e3)
                nc.scalar.activation(
                    e3[0:113, 0:392], st["s3"][cc][0:113, 0:392], AF.Exp, scale=SCALE23
                )

        def pair_scores(b, p2, st):
            pair_scores_s1(b, p2, st)
            pair_scores_s23(b, p2, st)

        def pair_pv1_mm(b, p2, st):
            st["pv1"] = []
            for hp in range(2):
                pv = psum("psPV1")
                st["pv1"].append(pv)
                for j, (k0, ksz) in enumerate(KEY1_T):
                    nc.tensor.matmul(
                        pv[0:65, 0:392],
                        v_ext[b][j][0:ksz, 65 * (2 * p2 + hp) : 65 * (2 * p2 + hp) + 65],
                        st["e1"][hp][j][0:ksz, 0:392],
                        start=(j == 0),
                        stop=(j == 3),
                    )

        def pair_pv1_norm_pre(b, p2, st):
            st["bc1"] = [norm_pre(st["pv1"][hp], 0) for hp in range(2)]

        def pair_pv1_norm_mul(b, p2, st):
            qt = p2
            for hp in range(2):
                qo = 64 * hp
                norm_mul(
                    st["pv1"][hp], 0, st["bc1"][hp],
                    aoT[b][qt][qo : qo + 64, 0:N_MT],
                )

        def pair_transp(b, p2, st):
            # [113, keys] -> [keys, 113] in 10 chunks, via identity matmul
            st["eT"] = []
            for half in range(2):
                trp = ppool.tile([128, 5 * TSP], BF, tag="P", name="psTr", bufs=8)
                for kk in range(5):
                    kt = 5 * half + kk
                    p0, psz = POS_T[kt]
                    nc.tensor.transpose(
                        trp[0:psz, TSP * kk : TSP * kk + 113],
                        st["e2"][0:113, p0 : p0 + psz],
                        ident[0:113, 0:113],
                    )
                eT = wpool.tile([128, 5 * TSP], BF, tag="e2T", bufs=4)
                st["eT"].append(eT)
                # evac on vector: scalar is exp-saturated in the pair slot
                nc.vector.tensor_copy(eT[:, 0 : 5 * TSP], trp[:, 0 : 5 * TSP])

        def pair_pv2(b, p2, st, do_memset=False):
            # both heads per matmul: lhsT = full transposed tile (garbage rows
            # 49:63 only pollute unused output rows), rhs = 129-wide v_ext
            # slice [v_h0 | ones | v_h1]; the ones col yields both heads'
            # sumexp at out col 64. 10 matmuls instead of 20.
            do_memset = do_memset or pv2_calls[0] < 2
            pv2_calls[0] += 1
            pv2 = psum("psPV2")
            for kt, (p0, psz) in enumerate(POS_T):
                eT = st["eT"][kt // 5]
                cof = TSP * (kt % 5)
                nc.tensor.matmul(
                    pv2[0:113, 0:129],
                    eT[0:psz, cof : cof + 113],
                    v_ext[b][kt][0:psz, 130 * p2 : 130 * p2 + 129],
                    start=(kt == 0),
                    stop=(kt == 9),
                )
            av = wpool.tile([128, 65], BF, tag="avx", bufs=2)
            st["av"] = av
            if do_memset:
                # two rotation slots; ts_muls only write cols 0:64, so the
                # ones column persists across later pairs
                nc.vector.memset(av[0:113, 64:65], 1.0)
            avr = wpool.tile([128, 1], F32, tag="avr", bufs=2)
            nc.vector.reciprocal(avr[0:113, 0:1], pv2[0:113, 64:65])
            nc.vector.tensor_scalar_mul(av[0:49, 0:64], pv2[0:49, 0:64], avr[0:49, 0:1])
            nc.vector.tensor_scalar_mul(
                av[64:113, 0:64], pv2[64:113, 65:129], avr[64:113, 0:1]
            )

        def pair_pv3_mm(b, p2, st):
            st["pv3"] = []
            for hp in range(2):
                for cc in range(2):
                    pv = psum("psPV3")
                    st["pv3"].append(pv)
                    nc.tensor.matmul(
                        pv[0:65, 0:392],
                        st["av"][64 * hp : 64 * hp + 49, 0:65],
                        st["e3"][cc][64 * hp : 64 * hp + 49, 0:392],
                        start=True,
                        stop=True,
                    )

        def pair_pv3_norm_pre(b, p2, st):
            st["bc3"] = [norm_pre(pv, 0) for pv in st["pv3"]]

        def pair_pv3_norm_mul(b, p2, st):
            qt = p2
            for i, pv in enumerate(st["pv3"]):
                hp, cc = divmod(i, 2)
                norm_mul(
                    pv,
                    0,
                    st["bc3"][i],
                    aoT[b][qt][64 * hp : 64 * hp + 64, N_MT + 392 * cc : N_MT + 392 * (cc + 1)],
                )

        def proj_unit(b, pt):
            p0, psz = POS_T[pt]
            ob = wpool.tile([128, C], BF, tag="osb")
            for c0, csz in CCHUNK:
                ps = psum("psPJ")
                for kt in range(6):
                    nc.tensor.matmul(
                        ps[0:psz, 0:csz],
                        aoT[b][kt][:, p0 : p0 + psz],
                        wp[kt][:, c0 : c0 + csz],
                        start=(kt == 0),
                        stop=(kt == 5),
                    )
                nc.vector.tensor_add(
                    ob[0:psz, c0 : c0 + csz], ps[0:psz, 0:csz], pb_bc[0:psz, c0 : c0 + csz]
                )
            nc.sync.dma_start(out_d[b, p0 : p0 + psz, :], ob[0:psz, :])

        def qk_pool_unit(b, m):
            q_unit(b, m)
            if m < 6:
                pool_ct(b, m)

        def qkv_units(b):
            units = []
            for m in range(12):
                units.append(lambda m=m: qk_pool_unit(b, m))
            for pt in range(10):
                units.append(lambda pt=pt: v_unit(b, pt))
            return units

        # ---- schedule ----
        # aoT[b] aliases qkT[b] q-tiles 0-5: by the time a pair's norms write
        # a tile, that pair's s1/s3 score matmuls (its only q readers) are
        # done -- the tile framework serializes the WAR. Saves 14KB/partition
        # and lets proj(b) units 2-9 defer into batch b+1's pair loop as PE
        # fillers (the last batch finally gets filler work).
        load_x(0)
        load_x(1)
        for u in qkv_units(0):
            u()

        prev = None
        for b in range(NB):
            aoT[b] = qkT[b][0:6]
            # fill order matters: proj(b-1) reads qkT[b-1] (same buf parity
            # as qkv(b+1)'s q_unit writes), so proj units must drain first.
            fill = []
            if b > 0:
                fill += [lambda pt=pt, pb=b - 1: proj_unit(pb, pt) for pt in range(2, 10)]
            if b + 1 < NB:
                fill += list(qkv_units(b + 1))
            if b + 2 < NB:
                load_x(b + 2)
            fi = 0
            # last batch has few fillers (proj of b-1 only): ration them
            # across iterations instead of exhausting them in the first two
            ration = 2 if b == NB - 1 else 10**9
            it_taken = [0]

            def take(n):
                nonlocal fi
                for _ in range(n):
                    if fi < len(fill) and it_taken[0] < ration:
                        fill[fi]()
                        fi += 1
                        it_taken[0] += 1

            # cross-pair software pipeline: iteration runs scores(+exps) of
            # pair p while the pv-stages of pair p-1 consume exps computed a
            # full iteration (~17us) earlier -- the scalar exp chain leaves
            # the PE critical path entirely.
            for p2 in range(6):
                st = {}
                it_taken[0] = 0
                if prev is None:
                    pair_scores(b, p2, st)
                    take(2)
                elif prev[1] == 5:
                    # batch boundary: pv-stage of (b-1, 5) woven with
                    # scores(b, 0). No takes until its pv3 norms land --
                    # the proj(b-1) fillers read the s-region they write
                    # (taking one earlier would deadlock the PE queue).
                    pb_, pp_, pst = prev
                    pair_pv1_mm(pb_, pp_, pst)
                    pair_transp(pb_, pp_, pst)
                    pair_pv1_norm_pre(pb_, pp_, pst)
                    pair_scores(b, p2, st)
                    pair_pv2(pb_, pp_, pst)
                    pair_pv1_norm_mul(pb_, pp_, pst)
                    proj_unit(pb_, 0)
                    pair_pv3_mm(pb_, pp_, pst)
                    pair_pv3_norm_pre(pb_, pp_, pst)
                    pair_pv3_norm_mul(pb_, pp_, pst)
                    proj_unit(pb_, 1)
                    take(4)
                elif fi < len(fill):
                    pb_, pp_, pst = prev
                    pair_scores(b, p2, st)
                    pair_pv1_mm(pb_, pp_, pst)
                    pair_transp(pb_, pp_, pst)
                    pair_pv1_norm_pre(pb_, pp_, pst)
                    take(1)
                    pair_pv2(pb_, pp_, pst)
                    pair_pv1_norm_mul(pb_, pp_, pst)
                    take(1)
                    pair_pv3_mm(pb_, pp_, pst)
                    pair_pv3_norm_pre(pb_, pp_, pst)
                    take(1)
                    pair_pv3_norm_mul(pb_, pp_, pst)
                    take(2)
                else:
                    # filler-starved (late b3): weave scores between pv2 and
                    # pv3 so the av vector chain hides under the score
                    # matmuls (same shape as the batch-boundary branch)
                    pb_, pp_, pst = prev
                    pair_pv1_mm(pb_, pp_, pst)
                    pair_transp(pb_, pp_, pst)
                    pair_pv1_norm_pre(pb_, pp_, pst)
                    pair_pv2(pb_, pp_, pst)
                    pair_pv1_norm_mul(pb_, pp_, pst)
                    pair_scores(b, p2, st)
                    pair_pv3_mm(pb_, pp_, pst)
                    pair_pv3_norm_pre(pb_, pp_, pst)
                    pair_pv3_norm_mul(pb_, pp_, pst)
                prev = (b, p2, st)
            it_taken[0] = -(10**9)  # drain remaining fillers unrationed
            take(len(fill))

        # drain: pv-stage of the final pair + last batch's proj
        pb_, pp_, pst = prev
        pair_pv1_mm(pb_, pp_, pst)
        pair_transp(pb_, pp_, pst)
        pair_pv1_norm_pre(pb_, pp_, pst)
        pair_pv2(pb_, pp_, pst)
        pair_pv1_norm_mul(pb_, pp_, pst)
        proj_unit(pb_, 0)
        pair_pv3_mm(pb_, pp_, pst)
        pair_pv3_norm_pre(pb_, pp_, pst)
        pair_pv3_norm_mul(pb_, pp_, pst)
        proj_unit(pb_, 1)
        for pt in range(2, 10):
            proj_unit(pb_, pt)

    nc.compile()
    return nc


def tc_ctx(nc):
    from contextlib import contextmanager

    @contextmanager
    def ctx():
        with tile.TileContext(nc) as tc, nc.allow_low_precision(reason="attn bf16"):
            with (
                tc.tile_pool(name="const", bufs=1) as cpool,
                tc.tile_pool(name="work", bufs=2) as wpool,
                tc.tile_pool(name="hold", bufs=1) as hpool,
                tc.tile_pool(name="psum", bufs=8, space="PSUM") as ppool,
            ):
                yield tc, cpool, wpool, hpool, ppool

    return ctx()


_PROGRAM = None


def _get_program():
    global _PROGRAM
    if _PROGRAM is None:
        _PROGRAM = build_program()
    return _PROGRAM


def _prep_inputs(x, qkv_w, qkv_b, proj_w, proj_b):
    bf = ml_dtypes.bfloat16
    x = np.asarray(x, dtype=np.float32)
    xT = np.ascontiguousarray(x.transpose(0, 2, 1)).astype(bf)  # [B, C, N]
    wqkT = np.ascontiguousarray(np.asarray(qkv_w, dtype=np.float32).T).astype(bf)
    wpjT = np.ascontiguousarray(np.asarray(proj_w, dtype=np.float32).T).astype(bf)
    qb = np.asarray(qkv_b, dtype=np.float32)
    vbb = np.broadcast_to(qb[2 * 768 :].astype(bf), (128, 768)).copy()
    bqkp = np.ascontiguousarray(qb[: 2 * 768].reshape(12, 128).T).astype(np.float32)
    pbb = np.broadcast_to(
        np.asarray(proj_b, dtype=np.float32).astype(bf), (128, 768)
    ).copy()
    in_maps = []
    for c in range(N_CORES):
        in_maps.append(
            {
                "xT": np.ascontiguousarray(xT[c * NB : (c + 1) * NB]),
                "wqkT": wqkT,
                "wpjT": wpjT,
                "vbb": vbb,
                "bqkp": bqkp,
                "pbb": pbb,
            }
        )
    return in_maps


def kernel(x, qkv_w, qkv_b, proj_w, proj_b, t_h=14, t_w=14, s_h=28, s_w=28, **kw):
    nc = _get_program()
    in_maps = _prep_inputs(x, qkv_w, qkv_b, proj_w, proj_b)
    res = bass_utils.run_bass_kernel_spmd(nc, in_maps, core_ids=list(range(N_CORES)))
    out = np.concatenate([res.results[c]["out"] for c in range(N_CORES)], axis=0)
    return np.asarray(out, dtype=np.float32)


if __name__ == "__main__":
    build_program()
    print("program built OK")



# revision 72
# speedup vs baseline: 1.0069x; 1.0050x over previous
"""AgentAttention Trainium2 kernel: 8-core data-parallel over batch.

v3: cross-PAIR software pipeline on top of v2's cross-batch fillers.
Iteration i runs scores+exps of head-pair p on PE/scalar while the pv
stages of pair p-1 consume exps computed a full iteration (~17us) earlier,
taking the scalar exp chain off the PE critical path (v2 stalled ~124
times/kernel on exp semaphores). Norm chains are split into pre (copy sumexp
row from psum -> recip -> gpsimd broadcast) and mul phases so the in-order
vector queue never head-of-line blocks on gpsimd. aoT aliases the dead q
tiles of qkT (the tile framework serializes the WAR), freeing 14KB/partition
and letting proj(b) units 2-9 defer into batch b+1 as PE fillers -- the
last batch (which has no next-batch qkv work) rations them across its
iterations. v3.1: bias-table DMAs issued before the big weight loads
(first q-evac stalled 5.8us on bqkp), bf16 output DMA with host upcast
(halves 15.6MB of out traffic; +0.2% quantization, rel err 3.5e-3 ->
4.5e-3 vs the 2e-2 gate), av ones-column memset once per rotation slot.
v3.2: filler-starved iterations (late b3) weave scores between pv2 and
pv3 so the av vector chain hides under score matmuls; wq DMA in q/k/v
column thirds. 605.7us baseline -> 548.0us.

Hard-won negative results (do not redo):
  - gpsimd cannot read PSUM (walrus rejects; custom-DVE recip from psum
    returns garbage on HW even though CoreSim passes).
  - gpsimd tensor_add for the pooling starves the norm broadcasts via
    library swaps: +140us.
  - dma_start on the scalar queue head-of-line blocks exps: +120us.
  - 2-bank psum claims with merged exps/evacs halve scalar ops but the
    4-deep rotation + coarser evac latency cost +56us net.
  - fp8 DoubleRow for qkv/proj GEMMs: e4m3's ~3% RMS error vs the 2e-2
    max-abs/absmax gate (= 3.5e-3 abs diff) fails at near-absmax outputs.
  - matmul PSUM dst at a sub-bank column offset (256B tried) raises a
    runtime exec fault; transposes tolerate sub-bank offsets, regular
    matmuls do not -- dsts must start at a bank boundary.
  - computing s2 directly transposed ([keys, agents] via a block-diagonal
    zero-padded agT) kills the 10 PE transposes + eT copies per pair but
    needs 10 one-bank psum claims and 10 small exps per pair: +83us net
    (claim-rotation stalls inside the scores stage dominate).
  - interleaving the tiny bqkp DMA between the wq half-loads (to exploit
    issue-order-cumulative completion waits) plus front-loading drain proj
    units: +100us -- do not reorder the const-DMA prologue.

Layouts (per core, 4 batches):
  xT      [4, 768, 1176] bf16  (c-major x)
  qkT     c-major q,k: 12 sbuf tiles [128, 1176] (tiles 0-5 = q, 6-11 = k);
          q tiles are overwritten in place by attention output (aoT alias)
  v_ext   pos-major v with per-head ones column (col 64): 10 tiles [128, 12*65]
  agT     pooled agent tokens (sums over 4x4 blocks), c-major [128, 49] x6
  aoT     = qkT[0:6] (bf16) -> proj -> out
Matmuls bf16, fp32 psum (uniform pool of 8 one-bank tiles [128,512]).
Softmax scale folded into ACT exp scale (0.125 stage1; 0.125/16 stages 2/3
-- agent tokens are pooled SUMS). qk bias via per-partition activation bias.
"""

import sys

sys.path.insert(0, "/opt/trn_rl_repo")

import numpy as np
import ml_dtypes

import concourse.bass as bass
import concourse.mybir as mybir
import concourse.tile as tile
from concourse import bacc, bass_utils
from concourse.masks import make_identity

BF = mybir.dt.bfloat16
F32 = mybir.dt.float32
AF = mybir.ActivationFunctionType

N_CORES = 8
B, N, C = 32, 1176, 768
NB = B // N_CORES
H, HD = 12, 64
N_MT, N_S = 392, 784
A = 49
SCALE1 = 0.125
SCALE23 = 0.125 / 16.0

POS_T = [(pt * 128, min(128, N - pt * 128)) for pt in range(10)]
KEY1_T = [(0, 128), (128, 128), (256, 128), (384, 8)]
NCHUNK = [(0, 392), (392, 392), (784, 392)]
CCHUNK = [(0, 512), (512, 256)]
TSP = 116  # transpose chunk col spacing (>=113, even)


def build_program():
    nc = bacc.Bacc("TRN2", debug=False, num_devices=N_CORES)

    xT_d = nc.dram_tensor("xT", [NB, C, N], BF, kind="ExternalInput").ap()
    wqkT_d = nc.dram_tensor("wqkT", [C, 3 * C], BF, kind="ExternalInput").ap()
    wpjT_d = nc.dram_tensor("wpjT", [C, C], BF, kind="ExternalInput").ap()
    vbb_d = nc.dram_tensor("vbb", [128, C], BF, kind="ExternalInput").ap()
    bqkp_d = nc.dram_tensor("bqkp", [128, 12], F32, kind="ExternalInput").ap()
    pbb_d = nc.dram_tensor("pbb", [128, C], BF, kind="ExternalInput").ap()
    # bf16 output (host upcasts): halves 15.6MB of out-DMA traffic and the
    # end-of-kernel drain; adds ~0.2% quantization, well inside the gate
    out_d = nc.dram_tensor("out", [NB, N, C], BF, kind="ExternalOutput").ap()

    with tc_ctx(nc) as (tc, cpool, wpool, hpool, ppool):
        # ---- one-time constants/weights ----
        wq = [
            cpool.tile([128, 3 * C], BF, tag=f"wq{i}", name=f"wq{i}") for i in range(6)
        ]
        wp = [cpool.tile([128, C], BF, tag=f"wp{i}", name=f"wp{i}") for i in range(6)]
        # tiny bias tables FIRST: the first q-evac needs bqkp and stalled
        # 5.8us queued behind the big weight loads
        bqkp = cpool.tile([128, 12], F32, tag="bqkp")
        nc.sync.dma_start(bqkp[:], bqkp_d[:])
        vb_bc = cpool.tile([128, C], BF, tag="vb_bc")
        nc.sync.dma_start(vb_bc[:], vbb_d[:])
        pb_bc = cpool.tile([128, C], BF, tag="pb_bc")
        nc.sync.dma_start(pb_bc[:], pbb_d[:])
        # split wq loads into q/k/v column thirds (same issue order as the
        # proven prologue, finer grain): the first six q_units depend only on
        # the q-column loads, so the opening ldweights waits ~3.6us not ~10us
        for cc in range(3):
            for i in range(6):
                nc.sync.dma_start(
                    wq[i][:, cc * C : (cc + 1) * C],
                    wqkT_d[128 * i : 128 * (i + 1), cc * C : (cc + 1) * C],
                )
        for i in range(6):
            nc.sync.dma_start(wp[i][:], wpjT_d[128 * i : 128 * (i + 1), :])
        ident = cpool.tile([128, 128], BF, tag="ident")
        make_identity(nc, ident[:])

        pv2_calls = [0]

        # per-batch tile handles (rotated via tags, bufs=2)
        xT = {}
        qkT = {}
        v_ext = {}
        agT = {}
        aoT = {}

        def psum(name):
            return ppool.tile([128, 512], F32, tag="P", name=name, bufs=8)

        def load_x(b):
            xT[b] = [
                hpool.tile([128, N], BF, tag=f"xT{i}", name=f"xT{i}", bufs=2)
                for i in range(6)
            ]
            eng = nc.scalar if b == 0 else nc.sync
            for i in range(6):
                eng.dma_start(xT[b][i][:], xT_d[b, 128 * i : 128 * (i + 1), :])

        def q_unit(b, m):
            # qkT[m] c-major [128, 1176] for q (m<6) / k (m>=6) rows
            if m == 0:
                qkT[b] = [None] * 12
            t = hpool.tile([128, N], BF, tag=f"qkT{m}", name=f"qkT{m}", bufs=2)
            qkT[b][m] = t
            for n0, nsz in NCHUNK:
                ps = psum("psQ")
                for kt in range(6):
                    nc.tensor.matmul(
                        ps[:, 0:nsz],
                        wq[kt][:, 128 * m : 128 * (m + 1)],
                        xT[b][kt][:, n0 : n0 + nsz],
                        start=(kt == 0),
                        stop=(kt == 5),
                    )
                # evac on scalar engine (gpsimd cannot read PSUM)
                nc.scalar.activation(
                    t[:, n0 : n0 + nsz],
                    ps[:, 0:nsz],
                    AF.Identity,
                    bias=bqkp[:, m : m + 1],
                )

        def v_unit(b, pt):
            # pos-major v_ext [psz, 12*65] with ones col at 64 of each head.
            # both c-chunks in one 2-bank claim -> ONE merged evac add
            p0, psz = POS_T[pt]
            if pt == 0:
                v_ext[b] = [None] * 10
            vt = hpool.tile([128, H * 65], BF, tag=f"vx{pt}", name=f"vx{pt}", bufs=2)
            v_ext[b][pt] = vt
            if b < 2:
                # two rotation slots; evac only writes the 64 v columns, so
                # ones persist across later batches
                nc.vector.memset(
                    vt[:].rearrange("p (h e) -> p h e", e=65)[:, :, 64:65], 1.0
                )
            for ci, (c0, csz) in enumerate(CCHUNK):
                ps = psum("psV")
                for kt in range(6):
                    nc.tensor.matmul(
                        ps[0:psz, 0:csz],
                        xT[b][kt][:, p0 : p0 + psz],
                        wq[kt][:, 2 * C + c0 : 2 * C + c0 + csz],
                        start=(kt == 0),
                        stop=(kt == 5),
                    )
                nh = csz // 64
                h0 = c0 // 64
                nc.vector.tensor_add(
                    vt[0:psz].rearrange("p (h e) -> p h e", e=65)[
                        :, h0 : h0 + nh, 0:64
                    ],
                    ps[0:psz, 0:csz].rearrange("p (h d) -> p h d", d=64),
                    vb_bc[0:psz, c0 : c0 + csz].rearrange("p (h d) -> p h d", d=64),
                )

        def pool_ct(b, ct):
            # sum 4x4 blocks of q_s -> agT (c-major). On VECTOR: gpsimd must
            # stay broadcast-only (lib swaps + in-order blocking starve the
            # norm-chain broadcasts otherwise)
            if ct == 0:
                agT[b] = []
            t1 = wpool.tile([128, 196], F32, tag="t1", bufs=1)
            qs = qkT[b][ct][:, N_MT:N]  # [128, 784], idx = i*28 + aj*4 + dj
            q4 = qs.rearrange("p (x dj) -> p x dj", dj=4)
            nc.vector.tensor_add(t1[:, 0:196], q4[:, :, 0:1], q4[:, :, 1:2])
            nc.vector.tensor_add(t1[:, 0:196], t1[:, 0:196], q4[:, :, 2:3])
            nc.vector.tensor_add(t1[:, 0:196], t1[:, 0:196], q4[:, :, 3:4])
            ag = hpool.tile([128, A], BF, tag=f"ag{ct}", name=f"ag{ct}", bufs=2)
            agT[b].append(ag)
            # t1 idx = 28*ai + 7*di + aj -> view (ai, aj, di)
            t4 = t1[:, 0:196].rearrange("p (ai di aj) -> p ai aj di", ai=7, di=4)
            t2 = wpool.tile([128, A], F32, tag="t2")
            nc.vector.tensor_add(t2[:, 0:A], t4[:, :, :, 0:1], t4[:, :, :, 1:2])
            nc.vector.tensor_add(t2[:, 0:A], t2[:, 0:A], t4[:, :, :, 2:3])
            nc.vector.tensor_add(ag[:, 0:A], t2[:, 0:A], t4[:, :, :, 3:4])

        def norm_pre(pv, c0):
            # recip of the psum sumexp row, broadcast to 64 partitions.
            # (custom-DVE recip reading PSUM directly returns garbage on HW;
            # stage the sumexp row through SBUF first.) Split from the mul so
            # the in-order vector queue never head-of-line blocks on gpsimd.
            se = wpool.tile([1, 392], F32, tag="se", bufs=2)
            nc.vector.tensor_copy(se[:, 0:392], pv[64:65, c0 : c0 + 392])
            rc = wpool.tile([1, 392], F32, tag="rc", bufs=2)
            nc.vector.reciprocal_approx_fast(out=rc[:, 0:392], in_=se[:, 0:392])
            bc = wpool.tile([64, 392], F32, tag="bc", bufs=6)
            nc.gpsimd.partition_broadcast(bc[:], rc[0:1, 0:392])
            return bc

        def norm_mul(pv, c0, bc, dst):
            nc.vector.tensor_mul(dst, pv[0:64, c0 : c0 + 392], bc[:])

        # ---- attention for one head pair, split into schedulable chunks ----
        def pair_scores_s1(b, p2, st):
            qt = p2
            # stage 1 scores first: [keys, queries] per head over 4 key chunks.
            # Claim order matches exp (= psum evacuation) order so the 8-bank
            # rotation never waits, and e1 (pv1's dep) is computed earliest.
            st["s1"] = []
            for hp in range(2):
                qo = 64 * hp
                chunks = []
                st["s1"].append(chunks)
                for k0, ksz in KEY1_T:
                    ps = psum("psS1")
                    chunks.append(ps)
                    nc.tensor.matmul(
                        ps[0:ksz, 0:392],
                        qkT[b][6 + qt][qo : qo + 64, k0 : k0 + ksz],
                        qkT[b][qt][qo : qo + 64, 0:N_MT],
                        start=True,
                        stop=True,
                    )
        def pair_scores_s23(b, p2, st):
            qt = p2
            # stage 2 scores: [49x2 packed, keys] over 3 chunks
            st["s2"] = []
            for n0, nsz in NCHUNK:
                ps = psum("psS2")
                st["s2"].append(ps)
                for hp in range(2):
                    qo = 64 * hp
                    nc.tensor.matmul(
                        ps[qo : qo + 49, 0:nsz],
                        agT[b][qt][qo : qo + 64, 0:A],
                        qkT[b][6 + qt][qo : qo + 64, n0 : n0 + nsz],
                        start=True,
                        stop=True,
                    )
            # stage 3 scores: [49x2 packed (agents), queries] over 2 chunks
            st["s3"] = []
            for cc in range(2):
                ps = psum("psS3")
                st["s3"].append(ps)
                for hp in range(2):
                    qo = 64 * hp
                    nc.tensor.matmul(
                        ps[qo : qo + 49, 0:392],
                        agT[b][qt][qo : qo + 64, 0:A],
                        qkT[b][qt][qo : qo + 64, N_MT + 392 * cc : N_MT + 392 * (cc + 1)],
                        start=True,
                        stop=True,
                    )
            # exps (scalar engine) in the same order as the score claims.
            # bufs sized for TWO pairs in flight (cross-pair pipeline)
            st["e1"] = []
            for hp in range(2):
                e1s = []
                st["e1"].append(e1s)
                for j, (k0, ksz) in enumerate(KEY1_T):
                    e1 = wpool.tile([128, 392], BF, tag="e1", name="e1", bufs=16)
                    e1s.append(e1)
                    nc.scalar.activation(
                        e1[0:ksz, 0:392],
                        st["s1"][hp][j][0:ksz, 0:392],
                        AF.Exp,
                        scale=SCALE1,
                    )
            e2 = wpool.tile([128, N], BF, tag="e2", bufs=2)
            st["e2"] = e2
            for j, (n0, nsz) in enumerate(NCHUNK):
                nc.scalar.activation(
                    e2[0:113, n0 : n0 + nsz],
                    st["s2"][j][0:113, 0:nsz],
                    AF.Exp,
                    scale=SCALE23,
                )
            st["e3"] = []
            for cc in range(2):
                e3 = wpool.tile([128, 392], BF, tag="e3", name="e3", bufs=4)
                st["e3"].append(e3)
                nc.scalar.activation(
                    e3[0:113, 0:392], st["s3"][cc][0:113, 0:392], AF.Exp, scale=SCALE23
                )

        def pair_scores(b, p2, st):
            pair_scores_s1(b, p2, st)
            pair_scores_s23(b, p2, st)

        def pair_pv1_mm(b, p2, st):
            st["pv1"] = []
            for hp in range(2):
                pv = psum("psPV1")
                st["pv1"].append(pv)
                for j, (k0, ksz) in enumerate(KEY1_T):
                    nc.tensor.matmul(
                        pv[0:65, 0:392],
                        v_ext[b][j][0:ksz, 65 * (2 * p2 + hp) : 65 * (2 * p2 + hp) + 65],
                        st["e1"][hp][j][0:ksz, 0:392],
                        start=(j == 0),
                        stop=(j == 3),
                    )

        def pair_pv1_norm_pre(b, p2, st):
            st["bc1"] = [norm_pre(st["pv1"][hp], 0) for hp in range(2)]

        def pair_pv1_norm_mul(b, p2, st):
            qt = p2
            for hp in range(2):
                qo = 64 * hp
                norm_mul(
                    st["pv1"][hp], 0, st["bc1"][hp],
                    aoT[b][qt][qo : qo + 64, 0:N_MT],
                )

        def pair_transp(b, p2, st):
            # [113, keys] -> [keys, 113] in 10 chunks, via identity matmul
            st["eT"] = []
            for half in range(2):
                trp = ppool.tile([128, 5 * TSP], BF, tag="P", name="psTr", bufs=8)
                for kk in range(5):
                    kt = 5 * half + kk
                    p0, psz = POS_T[kt]
                    nc.tensor.transpose(
                        trp[0:psz, TSP * kk : TSP * kk + 113],
                        st["e2"][0:113, p0 : p0 + psz],
                        ident[0:113, 0:113],
                    )
                eT = wpool.tile([128, 5 * TSP], BF, tag="e2T", bufs=4)
                st["eT"].append(eT)
                # evac on vector: scalar is exp-saturated in the pair slot
                nc.vector.tensor_copy(eT[:, 0 : 5 * TSP], trp[:, 0 : 5 * TSP])

        def pair_pv2(b, p2, st, do_memset=False):
            # both heads per matmul: lhsT = full transposed tile (garbage rows
            # 49:63 only pollute unused output rows), rhs = 129-wide v_ext
            # slice [v_h0 | ones | v_h1]; the ones col yields both heads'
            # sumexp at out col 64. 10 matmuls instead of 20.
            do_memset = do_memset or pv2_calls[0] < 2
            pv2_calls[0] += 1
            pv2 = psum("psPV2")
            for kt, (p0, psz) in enumerate(POS_T):
                eT = st["eT"][kt // 5]
                cof = TSP * (kt % 5)
                nc.tensor.matmul(
                    pv2[0:113, 0:129],
                    eT[0:psz, cof : cof + 113],
                    v_ext[b][kt][0:psz, 130 * p2 : 130 * p2 + 129],
                    start=(kt == 0),
                    stop=(kt == 9),
                )
            av = wpool.tile([128, 65], BF, tag="avx", bufs=2)
            st["av"] = av
            if do_memset:
                # two rotation slots; ts_muls only write cols 0:64, so the
                # ones column persists across later pairs
                nc.vector.memset(av[0:113, 64:65], 1.0)
            avr = wpool.tile([128, 1], F32, tag="avr", bufs=2)
            nc.vector.reciprocal(avr[0:113, 0:1], pv2[0:113, 64:65])
            nc.vector.tensor_scalar_mul(av[0:49, 0:64], pv2[0:49, 0:64], avr[0:49, 0:1])
            nc.vector.tensor_scalar_mul(
                av[64:113, 0:64], pv2[64:113, 65:129], avr[64:113, 0:1]
            )

        def pair_pv3_mm(b, p2, st):
            st["pv3"] = []
            for hp in range(2):
                for cc in range(2):
                    pv = psum("psPV3")
                    st["pv3"].append(pv)
                    nc.tensor.matmul(
                        pv[0:65, 0:392],
                        st["av"][64 * hp : 64 * hp + 49, 0:65],
                        st["e3"][cc][64 * hp : 64 * hp + 49, 0:392],
                        start=True,
                        stop=True,
                    )

        def pair_pv3_norm_pre(b, p2, st):
            st["bc3"] = [norm_pre(pv, 0) for pv in st["pv3"]]

        def pair_pv3_norm_mul(b, p2, st):
            qt = p2
            for i, pv in enumerate(st["pv3"]):
                hp, cc = divmod(i, 2)
                norm_mul(
                    pv,
                    0,
                    st["bc3"][i],
                    aoT[b][qt][64 * hp : 64 * hp + 64, N_MT + 392 * cc : N_MT + 392 * (cc + 1)],
                )

        def proj_unit(b, pt):
            p0, psz = POS_T[pt]
            ob = wpool.tile([128, C], BF, tag="osb")
            for c0, csz in CCHUNK:
                ps = psum("psPJ")
                for kt in range(6):
                    nc.tensor.matmul(
                        ps[0:psz, 0:csz],
                        aoT[b][kt][:, p0 : p0 + psz],
                        wp[kt][:, c0 : c0 + csz],
                        start=(kt == 0),
                        stop=(kt == 5),
                    )
                nc.vector.tensor_add(
                    ob[0:psz, c0 : c0 + csz], ps[0:psz, 0:csz], pb_bc[0:psz, c0 : c0 + csz]
                )
            nc.sync.dma_start(out_d[b, p0 : p0 + psz, :], ob[0:psz, :])

        def qk_pool_unit(b, m):
            q_unit(b, m)
            if m < 6:
                pool_ct(b, m)

        def qkv_units(b):
            units = []
            for m in range(12):
                units.append(lambda m=m: qk_pool_unit(b, m))
            for pt in range(10):
                units.append(lambda pt=pt: v_unit(b, pt))
            return units

        # ---- schedule ----
        # aoT[b] aliases qkT[b] q-tiles 0-5: by the time a pair's norms write
        # a tile, that pair's s1/s3 score matmuls (its only q readers) are
        # done -- the tile framework serializes the WAR. Saves 14KB/partition
        # and lets proj(b) units 2-9 defer into batch b+1's pair loop as PE
        # fillers (the last batch finally gets filler work).
        load_x(0)
        load_x(1)
        for u in qkv_units(0):
            u()

        prev = None
        for b in range(NB):
            aoT[b] = qkT[b][0:6]
            # fill order matters: proj(b-1) reads qkT[b-1] (same buf parity
            # as qkv(b+1)'s q_unit writes), so proj units must drain first.
            fill = []
            if b > 0:
                fill += [lambda pt=pt, pb=b - 1: proj_unit(pb, pt) for pt in range(2, 10)]
            if b + 1 < NB:
                fill += list(qkv_units(b + 1))
            if b + 2 < NB:
                load_x(b + 2)
            fi = 0
            # last batch has few fillers (proj of b-1 only): ration them
            # across iterations instead of exhausting them in the first two
            ration = 2 if b == NB - 1 else 10**9
            it_taken = [0]

            def take(n):
                nonlocal fi
                for _ in range(n):
                    if fi < len(fill) and it_taken[0] < ration:
                        fill[fi]()
                        fi += 1
                        it_taken[0] += 1

            # cross-pair software pipeline: iteration runs scores(+exps) of
            # pair p while the pv-stages of pair p-1 consume exps computed a
            # full iteration (~17us) earlier -- the scalar exp chain leaves
            # the PE critical path entirely.
            for p2 in range(6):
                st = {}
                it_taken[0] = 0
                if prev is None:
                    pair_scores(b, p2, st)
                    take(2)
                elif prev[1] == 5:
                    # batch boundary: pv-stage of (b-1, 5) woven with
                    # scores(b, 0). No takes until its pv3 norms land --
                    # the proj(b-1) fillers read the s-region they write
                    # (taking one earlier would deadlock the PE queue).
                    pb_, pp_, pst = prev
                    pair_pv1_mm(pb_, pp_, pst)
                    pair_transp(pb_, pp_, pst)
                    pair_pv1_norm_pre(pb_, pp_, pst)
                    pair_scores(b, p2, st)
                    pair_pv2(pb_, pp_, pst)
                    pair_pv1_norm_mul(pb_, pp_, pst)
                    proj_unit(pb_, 0)
                    pair_pv3_mm(pb_, pp_, pst)
                    pair_pv3_norm_pre(pb_, pp_, pst)
                    pair_pv3_norm_mul(pb_, pp_, pst)
                    proj_unit(pb_, 1)
                    take(4)
                elif fi < len(fill):
                    pb_, pp_, pst = prev
                    pair_scores(b, p2, st)
                    pair_pv1_mm(pb_, pp_, pst)
                    pair_transp(pb_, pp_, pst)
                    pair_pv1_norm_pre(pb_, pp_, pst)
                    take(1)
                    pair_pv2(pb_, pp_, pst)
                    pair_pv1_norm_mul(pb_, pp_, pst)
                    take(1)
                    pair_pv3_mm(pb_, pp_, pst)
                    pair_pv3_norm_pre(pb_, pp_, pst)
                    take(1)
                    pair_pv3_norm_mul(pb_, pp_, pst)
                    take(2)
                else:
                    # filler-starved (late b3): weave scores between pv2 and
                    # pv3 so the av vector chain hides under the score
                    # matmuls (same shape as the batch-boundary branch)
                    pb_, pp_, pst = prev
                    pair_pv1_mm(pb_, pp_, pst)
                    pair_transp(pb_, pp_, pst)
                    pair_pv1_norm_pre(pb_, pp_, pst)
                    pair_pv2(pb_, pp_, pst)
                    pair_pv1_norm_mul(pb_, pp_, pst)
                    pair_scores(b, p2, st)
                    pair_pv3_mm(pb_, pp_, pst)
                    pair_pv3_norm_pre(pb_, pp_, pst)
                    pair_pv3_norm_mul(pb_, pp_, pst)
                prev = (b, p2, st)
            it_taken[0] = -(10**9)  # drain remaining fillers unrationed
            take(len(fill))

        # drain: pv-stage of the final pair + last batch's proj
        pb_, pp_, pst = prev
        pair_pv1_mm(pb_, pp_, pst)
        pair_transp(pb_, pp_, pst)
        pair_pv1_norm_pre(pb_, pp_, pst)
        pair_pv2(pb_, pp_, pst)
        pair_pv1_norm_mul(pb_, pp_, pst)
        proj_unit(pb_, 0)
        pair_pv3_mm(pb_, pp_, pst)
        pair_pv3_norm_pre(pb_, pp_, pst)
        pair_pv3_norm_mul(pb_, pp_, pst)
        proj_unit(pb_, 1)
        for pt in range(2, 10):
            proj_unit(pb_, pt)

    nc.compile()
    return nc


def tc_ctx(nc):
    from contextlib import contextmanager

    @contextmanager
    def ctx():
        with tile.TileContext(nc) as tc, nc.allow_low_precision(reason="attn bf16"):
            with (
                tc.tile_pool(name="const", bufs=1) as cpool,
                tc.tile_pool(name="work", bufs=2) as wpool,
                tc.tile_pool(name="hold", bufs=1) as hpool,
                tc.tile_pool(name="psum", bufs=8, space="PSUM") as ppool,
            ):
                yield tc, cpool, wpool, hpool, ppool

    return ctx()


_PROGRAM = None


def _get_program():
    global _PROGRAM
    if _PROGRAM is None:
        _PROGRAM = build_program()
    return _PROGRAM


def _prep_inputs(x, qkv_w, qkv_b, proj_w, proj_b):
    bf = ml_dtypes.bfloat16
    x = np.asarray(x, dtype=np.float32)
    xT = np.ascontiguousarray(x.transpose(0, 2, 1)).astype(bf)  # [B, C, N]
    wqkT = np.ascontiguousarray(np.asarray(qkv_w, dtype=np.float32).T).astype(bf)
    wpjT = np.ascontiguousarray(np.asarray(proj_w, dtype=np.float32).T).astype(bf)
    qb = np.asarray(qkv_b, dtype=np.float32)
    vbb = np.broadcast_to(qb[2 * 768 :].astype(bf), (128, 768)).copy()
    bqkp = np.ascontiguousarray(qb[: 2 * 768].reshape(12, 128).T).astype(np.float32)
    pbb = np.broadcast_to(
        np.asarray(proj_b, dtype=np.float32).astype(bf), (128, 768)
    ).copy()
    in_maps = []
    for c in range(N_CORES):
        in_maps.append(
            {
                "xT": np.ascontiguousarray(xT[c * NB : (c + 1) * NB]),
                "wqkT": wqkT,
                "wpjT": wpjT,
                "vbb": vbb,
                "bqkp": bqkp,
                "pbb": pbb,
            }
        )
    return in_maps


def kernel(x, qkv_w, qkv_b, proj_w, proj_b, t_h=14, t_w=14, s_h=28, s_w=28, **kw):
    nc = _get_program()
    in_maps = _prep_inputs(x, qkv_w, qkv_b, proj_w, proj_b)
    res = bass_utils.run_bass_kernel_spmd(nc, in_maps, core_ids=list(range(N_CORES)))
    out = np.concatenate([res.results[c]["out"] for c in range(N_CORES)], axis=0)
    return np.asarray(out, dtype=np.float32)


if __name__ == "__main__":
    build_program()
    print("program built OK")



# revision 73
# speedup vs baseline: 1.0097x; 1.0028x over previous
"""AgentAttention Trainium2 kernel: 8-core data-parallel over batch.

v3: cross-PAIR software pipeline on top of v2's cross-batch fillers.
Iteration i runs scores+exps of head-pair p on PE/scalar while the pv
stages of pair p-1 consume exps computed a full iteration (~17us) earlier,
taking the scalar exp chain off the PE critical path (v2 stalled ~124
times/kernel on exp semaphores). Norm chains are split into pre (copy sumexp
row from psum -> recip -> gpsimd broadcast) and mul phases so the in-order
vector queue never head-of-line blocks on gpsimd. aoT aliases the dead q
tiles of qkT (the tile framework serializes the WAR), freeing 14KB/partition
and letting proj(b) units 2-9 defer into batch b+1 as PE fillers -- the
last batch (which has no next-batch qkv work) rations them across its
iterations. v3.1: bias-table DMAs issued before the big weight loads
(first q-evac stalled 5.8us on bqkp), bf16 output DMA with host upcast
(halves 15.6MB of out traffic; +0.2% quantization, rel err 3.5e-3 ->
4.5e-3 vs the 2e-2 gate), av ones-column memset once per rotation slot.
v3.2: filler-starved iterations (late b3) weave scores between pv2 and
pv3 so the av vector chain hides under score matmuls; wq DMA in q/k/v
column thirds. 605.7us baseline -> 548.0us.

Hard-won negative results (do not redo):
  - gpsimd cannot read PSUM (walrus rejects; custom-DVE recip from psum
    returns garbage on HW even though CoreSim passes).
  - gpsimd tensor_add for the pooling starves the norm broadcasts via
    library swaps: +140us.
  - dma_start on the scalar queue head-of-line blocks exps: +120us.
  - 2-bank psum claims with merged exps/evacs halve scalar ops but the
    4-deep rotation + coarser evac latency cost +56us net.
  - fp8 DoubleRow for qkv/proj GEMMs: e4m3's ~3% RMS error vs the 2e-2
    max-abs/absmax gate (= 3.5e-3 abs diff) fails at near-absmax outputs.
  - matmul PSUM dst at a sub-bank column offset (256B tried) raises a
    runtime exec fault; transposes tolerate sub-bank offsets, regular
    matmuls do not -- dsts must start at a bank boundary.
  - computing s2 directly transposed ([keys, agents] via a block-diagonal
    zero-padded agT) kills the 10 PE transposes + eT copies per pair but
    needs 10 one-bank psum claims and 10 small exps per pair: +83us net
    (claim-rotation stalls inside the scores stage dominate).
  - interleaving the tiny bqkp DMA between the wq half-loads (to exploit
    issue-order-cumulative completion waits) plus front-loading drain proj
    units: +100us -- do not reorder the const-DMA prologue.

Layouts (per core, 4 batches):
  xT      [4, 768, 1176] bf16  (c-major x)
  qkT     c-major q,k: 12 sbuf tiles [128, 1176] (tiles 0-5 = q, 6-11 = k);
          q tiles are overwritten in place by attention output (aoT alias)
  v_ext   pos-major v with per-head ones column (col 64): 10 tiles [128, 12*65]
  agT     pooled agent tokens (sums over 4x4 blocks), c-major [128, 49] x6
  aoT     = qkT[0:6] (bf16) -> proj -> out
Matmuls bf16, fp32 psum (uniform pool of 8 one-bank tiles [128,512]).
Softmax scale folded into ACT exp scale (0.125 stage1; 0.125/16 stages 2/3
-- agent tokens are pooled SUMS). qk bias via per-partition activation bias.
"""

import sys

sys.path.insert(0, "/opt/trn_rl_repo")

import numpy as np
import ml_dtypes

import concourse.bass as bass
import concourse.mybir as mybir
import concourse.tile as tile
from concourse import bacc, bass_utils
from concourse.masks import make_identity

BF = mybir.dt.bfloat16
F32 = mybir.dt.float32
AF = mybir.ActivationFunctionType

N_CORES = 8
B, N, C = 32, 1176, 768
NB = B // N_CORES
H, HD = 12, 64
N_MT, N_S = 392, 784
A = 49
SCALE1 = 0.125
SCALE23 = 0.125 / 16.0

POS_T = [(pt * 128, min(128, N - pt * 128)) for pt in range(10)]
KEY1_T = [(0, 128), (128, 128), (256, 128), (384, 8)]
NCHUNK = [(0, 392), (392, 392), (784, 392)]
CCHUNK = [(0, 512), (512, 256)]
TSP = 116  # transpose chunk col spacing (>=113, even)


def build_program():
    nc = bacc.Bacc("TRN2", debug=False, num_devices=N_CORES)

    xT_d = nc.dram_tensor("xT", [NB, C, N], BF, kind="ExternalInput").ap()
    wqkT_d = nc.dram_tensor("wqkT", [C, 3 * C], BF, kind="ExternalInput").ap()
    wpjT_d = nc.dram_tensor("wpjT", [C, C], BF, kind="ExternalInput").ap()
    vbb_d = nc.dram_tensor("vbb", [128, C], BF, kind="ExternalInput").ap()
    bqkp_d = nc.dram_tensor("bqkp", [128, 12], F32, kind="ExternalInput").ap()
    pbb_d = nc.dram_tensor("pbb", [128, C], BF, kind="ExternalInput").ap()
    # bf16 output (host upcasts): halves 15.6MB of out-DMA traffic and the
    # end-of-kernel drain; adds ~0.2% quantization, well inside the gate
    out_d = nc.dram_tensor("out", [NB, N, C], BF, kind="ExternalOutput").ap()

    with tc_ctx(nc) as (tc, cpool, wpool, hpool, ppool):
        # ---- one-time constants/weights ----
        wq = [
            cpool.tile([128, 3 * C], BF, tag=f"wq{i}", name=f"wq{i}") for i in range(6)
        ]
        wp = [cpool.tile([128, C], BF, tag=f"wp{i}", name=f"wp{i}") for i in range(6)]
        # tiny bias tables FIRST: the first q-evac needs bqkp and stalled
        # 5.8us queued behind the big weight loads
        bqkp = cpool.tile([128, 12], F32, tag="bqkp")
        nc.sync.dma_start(bqkp[:], bqkp_d[:])
        vb_bc = cpool.tile([128, C], BF, tag="vb_bc")
        nc.sync.dma_start(vb_bc[:], vbb_d[:])
        pb_bc = cpool.tile([128, C], BF, tag="pb_bc")
        nc.sync.dma_start(pb_bc[:], pbb_d[:])
        # split wq loads into q/k/v column thirds (same issue order as the
        # proven prologue, finer grain): the first six q_units depend only on
        # the q-column loads, so the opening ldweights waits ~3.6us not ~10us
        for cc in range(3):
            for i in range(6):
                nc.sync.dma_start(
                    wq[i][:, cc * C : (cc + 1) * C],
                    wqkT_d[128 * i : 128 * (i + 1), cc * C : (cc + 1) * C],
                )
        for i in range(6):
            nc.sync.dma_start(wp[i][:], wpjT_d[128 * i : 128 * (i + 1), :])
        ident = cpool.tile([128, 128], BF, tag="ident")
        make_identity(nc, ident[:])

        pv2_calls = [0]

        # per-batch tile handles (rotated via tags, bufs=2)
        xT = {}
        qkT = {}
        v_ext = {}
        agT = {}
        aoT = {}

        def psum(name):
            return ppool.tile([128, 512], F32, tag="P", name=name, bufs=8)

        def load_x(b):
            xT[b] = [
                hpool.tile([128, N], BF, tag=f"xT{i}", name=f"xT{i}", bufs=2)
                for i in range(6)
            ]
            eng = nc.scalar if b == 0 else nc.sync
            for i in range(6):
                eng.dma_start(xT[b][i][:], xT_d[b, 128 * i : 128 * (i + 1), :])

        def q_unit(b, m):
            # qkT[m] c-major [128, 1176] for q (m<6) / k (m>=6) rows
            if m == 0:
                qkT[b] = [None] * 12
            t = hpool.tile([128, N], BF, tag=f"qkT{m}", name=f"qkT{m}", bufs=2)
            qkT[b][m] = t
            for n0, nsz in NCHUNK:
                ps = psum("psQ")
                for kt in range(6):
                    nc.tensor.matmul(
                        ps[:, 0:nsz],
                        wq[kt][:, 128 * m : 128 * (m + 1)],
                        xT[b][kt][:, n0 : n0 + nsz],
                        start=(kt == 0),
                        stop=(kt == 5),
                    )
                # evac on scalar engine (gpsimd cannot read PSUM)
                nc.scalar.activation(
                    t[:, n0 : n0 + nsz],
                    ps[:, 0:nsz],
                    AF.Identity,
                    bias=bqkp[:, m : m + 1],
                )

        def v_unit(b, pt):
            # pos-major v_ext [psz, 12*65] with ones col at 64 of each head.
            # both c-chunks in one 2-bank claim -> ONE merged evac add
            p0, psz = POS_T[pt]
            if pt == 0:
                v_ext[b] = [None] * 10
            vt = hpool.tile([128, H * 65], BF, tag=f"vx{pt}", name=f"vx{pt}", bufs=2)
            v_ext[b][pt] = vt
            if b < 2:
                # two rotation slots; evac only writes the 64 v columns, so
                # ones persist across later batches
                nc.vector.memset(
                    vt[:].rearrange("p (h e) -> p h e", e=65)[:, :, 64:65], 1.0
                )
            for ci, (c0, csz) in enumerate(CCHUNK):
                ps = psum("psV")
                for kt in range(6):
                    nc.tensor.matmul(
                        ps[0:psz, 0:csz],
                        xT[b][kt][:, p0 : p0 + psz],
                        wq[kt][:, 2 * C + c0 : 2 * C + c0 + csz],
                        start=(kt == 0),
                        stop=(kt == 5),
                    )
                nh = csz // 64
                h0 = c0 // 64
                nc.vector.tensor_add(
                    vt[0:psz].rearrange("p (h e) -> p h e", e=65)[
                        :, h0 : h0 + nh, 0:64
                    ],
                    ps[0:psz, 0:csz].rearrange("p (h d) -> p h d", d=64),
                    vb_bc[0:psz, c0 : c0 + csz].rearrange("p (h d) -> p h d", d=64),
                )

        def pool_ct(b, ct):
            # sum 4x4 blocks of q_s -> agT (c-major). On VECTOR: gpsimd must
            # stay broadcast-only (lib swaps + in-order blocking starve the
            # norm-chain broadcasts otherwise)
            if ct == 0:
                agT[b] = []
            t1 = wpool.tile([128, 196], F32, tag="t1", bufs=1)
            qs = qkT[b][ct][:, N_MT:N]  # [128, 784], idx = i*28 + aj*4 + dj
            q4 = qs.rearrange("p (x dj) -> p x dj", dj=4)
            nc.vector.tensor_add(t1[:, 0:196], q4[:, :, 0:1], q4[:, :, 1:2])
            nc.vector.tensor_add(t1[:, 0:196], t1[:, 0:196], q4[:, :, 2:3])
            nc.vector.tensor_add(t1[:, 0:196], t1[:, 0:196], q4[:, :, 3:4])
            ag = hpool.tile([128, A], BF, tag=f"ag{ct}", name=f"ag{ct}", bufs=2)
            agT[b].append(ag)
            # t1 idx = 28*ai + 7*di + aj -> view (ai, aj, di)
            t4 = t1[:, 0:196].rearrange("p (ai di aj) -> p ai aj di", ai=7, di=4)
            t2 = wpool.tile([128, A], F32, tag="t2")
            nc.vector.tensor_add(t2[:, 0:A], t4[:, :, :, 0:1], t4[:, :, :, 1:2])
            nc.vector.tensor_add(t2[:, 0:A], t2[:, 0:A], t4[:, :, :, 2:3])
            nc.vector.tensor_add(ag[:, 0:A], t2[:, 0:A], t4[:, :, :, 3:4])

        def norm_pre(pv, c0):
            # recip of the psum sumexp row, broadcast to 64 partitions.
            # (custom-DVE recip reading PSUM directly returns garbage on HW;
            # stage the sumexp row through SBUF first.) Split from the mul so
            # the in-order vector queue never head-of-line blocks on gpsimd.
            se = wpool.tile([1, 392], F32, tag="se", bufs=2)
            nc.vector.tensor_copy(se[:, 0:392], pv[64:65, c0 : c0 + 392])
            rc = wpool.tile([1, 392], F32, tag="rc", bufs=2)
            nc.vector.reciprocal_approx_fast(out=rc[:, 0:392], in_=se[:, 0:392])
            bc = wpool.tile([64, 392], F32, tag="bc", bufs=6)
            nc.gpsimd.partition_broadcast(bc[:], rc[0:1, 0:392])
            return bc

        def norm_mul(pv, c0, bc, dst):
            nc.vector.tensor_mul(dst, pv[0:64, c0 : c0 + 392], bc[:])

        # ---- attention for one head pair, split into schedulable chunks ----
        def pair_scores_s1(b, p2, st):
            qt = p2
            # stage 1 scores first: [keys, queries] per head over 4 key chunks.
            # Claim order matches exp (= psum evacuation) order so the 8-bank
            # rotation never waits, and e1 (pv1's dep) is computed earliest.
            st["s1"] = []
            for hp in range(2):
                qo = 64 * hp
                chunks = []
                st["s1"].append(chunks)
                for k0, ksz in KEY1_T:
                    ps = psum("psS1")
                    chunks.append(ps)
                    nc.tensor.matmul(
                        ps[0:ksz, 0:392],
                        qkT[b][6 + qt][qo : qo + 64, k0 : k0 + ksz],
                        qkT[b][qt][qo : qo + 64, 0:N_MT],
                        start=True,
                        stop=True,
                    )
        def pair_scores_s23(b, p2, st):
            qt = p2
            # stage 2 scores: [49x2 packed, keys] over 3 chunks
            st["s2"] = []
            for n0, nsz in NCHUNK:
                ps = psum("psS2")
                st["s2"].append(ps)
                for hp in range(2):
                    qo = 64 * hp
                    nc.tensor.matmul(
                        ps[qo : qo + 49, 0:nsz],
                        agT[b][qt][qo : qo + 64, 0:A],
                        qkT[b][6 + qt][qo : qo + 64, n0 : n0 + nsz],
                        start=True,
                        stop=True,
                    )
            # stage 3 scores: [49x2 packed (agents), queries] over 2 chunks
            st["s3"] = []
            for cc in range(2):
                ps = psum("psS3")
                st["s3"].append(ps)
                for hp in range(2):
                    qo = 64 * hp
                    nc.tensor.matmul(
                        ps[qo : qo + 49, 0:392],
                        agT[b][qt][qo : qo + 64, 0:A],
                        qkT[b][qt][qo : qo + 64, N_MT + 392 * cc : N_MT + 392 * (cc + 1)],
                        start=True,
                        stop=True,
                    )
            # exps (scalar engine) in the same order as the score claims.
            # bufs sized for TWO pairs in flight (cross-pair pipeline)
            st["e1"] = []
            for hp in range(2):
                e1s = []
                st["e1"].append(e1s)
                for j, (k0, ksz) in enumerate(KEY1_T):
                    e1 = wpool.tile([128, 392], BF, tag="e1", name="e1", bufs=16)
                    e1s.append(e1)
                    nc.scalar.activation(
                        e1[0:ksz, 0:392],
                        st["s1"][hp][j][0:ksz, 0:392],
                        AF.Exp,
                        scale=SCALE1,
                    )
            e2 = wpool.tile([128, N], BF, tag="e2", bufs=2)
            st["e2"] = e2
            for j, (n0, nsz) in enumerate(NCHUNK):
                nc.scalar.activation(
                    e2[0:113, n0 : n0 + nsz],
                    st["s2"][j][0:113, 0:nsz],
                    AF.Exp,
                    scale=SCALE23,
                )
            st["e3"] = []
            for cc in range(2):
                e3 = wpool.tile([128, 392], BF, tag="e3", name="e3", bufs=4)
                st["e3"].append(e3)
                nc.scalar.activation(
                    e3[0:113, 0:392], st["s3"][cc][0:113, 0:392], AF.Exp, scale=SCALE23
                )

        def pair_scores(b, p2, st):
            pair_scores_s1(b, p2, st)
            pair_scores_s23(b, p2, st)

        def pair_pv1_mm(b, p2, st):
            st["pv1"] = []
            for hp in range(2):
                pv = psum("psPV1")
                st["pv1"].append(pv)
                for j, (k0, ksz) in enumerate(KEY1_T):
                    nc.tensor.matmul(
                        pv[0:65, 0:392],
                        v_ext[b][j][0:ksz, 65 * (2 * p2 + hp) : 65 * (2 * p2 + hp) + 65],
                        st["e1"][hp][j][0:ksz, 0:392],
                        start=(j == 0),
                        stop=(j == 3),
                    )

        def pair_pv1_norm_pre(b, p2, st):
            st["bc1"] = [norm_pre(st["pv1"][hp], 0) for hp in range(2)]

        def pair_pv1_norm_mul(b, p2, st):
            qt = p2
            for hp in range(2):
                qo = 64 * hp
                norm_mul(
                    st["pv1"][hp], 0, st["bc1"][hp],
                    aoT[b][qt][qo : qo + 64, 0:N_MT],
                )

        def pair_transp(b, p2, st):
            # [113, keys] -> [keys, 113] in 10 chunks, via identity matmul
            st["eT"] = []
            for half in range(2):
                trp = ppool.tile([128, 5 * TSP], BF, tag="P", name="psTr", bufs=8)
                for kk in range(5):
                    kt = 5 * half + kk
                    p0, psz = POS_T[kt]
                    nc.tensor.transpose(
                        trp[0:psz, TSP * kk : TSP * kk + 113],
                        st["e2"][0:113, p0 : p0 + psz],
                        ident[0:113, 0:113],
                    )
                eT = wpool.tile([128, 5 * TSP], BF, tag="e2T", bufs=4)
                st["eT"].append(eT)
                # evac on vector: scalar is exp-saturated in the pair slot
                nc.vector.tensor_copy(eT[:, 0 : 5 * TSP], trp[:, 0 : 5 * TSP])

        def pair_pv2(b, p2, st, do_memset=False):
            # both heads per matmul: lhsT = full transposed tile (garbage rows
            # 49:63 only pollute unused output rows), rhs = 129-wide v_ext
            # slice [v_h0 | ones | v_h1]; the ones col yields both heads'
            # sumexp at out col 64. 10 matmuls instead of 20.
            do_memset = do_memset or pv2_calls[0] < 2
            pv2_calls[0] += 1
            pv2 = psum("psPV2")
            for kt, (p0, psz) in enumerate(POS_T):
                eT = st["eT"][kt // 5]
                cof = TSP * (kt % 5)
                nc.tensor.matmul(
                    pv2[0:113, 0:129],
                    eT[0:psz, cof : cof + 113],
                    v_ext[b][kt][0:psz, 130 * p2 : 130 * p2 + 129],
                    start=(kt == 0),
                    stop=(kt == 9),
                )
            av = wpool.tile([128, 65], BF, tag="avx", bufs=2)
            st["av"] = av
            if do_memset:
                # two rotation slots; ts_muls only write cols 0:64, so the
                # ones column persists across later pairs
                nc.vector.memset(av[0:113, 64:65], 1.0)
            avr = wpool.tile([128, 1], F32, tag="avr", bufs=2)
            nc.vector.reciprocal(avr[0:113, 0:1], pv2[0:113, 64:65])
            nc.vector.tensor_scalar_mul(av[0:49, 0:64], pv2[0:49, 0:64], avr[0:49, 0:1])
            nc.vector.tensor_scalar_mul(
                av[64:113, 0:64], pv2[64:113, 65:129], avr[64:113, 0:1]
            )

        def pair_pv3_mm(b, p2, st):
            st["pv3"] = []
            for hp in range(2):
                for cc in range(2):
                    pv = psum("psPV3")
                    st["pv3"].append(pv)
                    nc.tensor.matmul(
                        pv[0:65, 0:392],
                        st["av"][64 * hp : 64 * hp + 49, 0:65],
                        st["e3"][cc][64 * hp : 64 * hp + 49, 0:392],
                        start=True,
                        stop=True,
                    )

        def pair_pv3_norm_pre(b, p2, st):
            st["bc3"] = [norm_pre(pv, 0) for pv in st["pv3"]]

        def pair_pv3_norm_mul(b, p2, st):
            qt = p2
            for i, pv in enumerate(st["pv3"]):
                hp, cc = divmod(i, 2)
                norm_mul(
                    pv,
                    0,
                    st["bc3"][i],
                    aoT[b][qt][64 * hp : 64 * hp + 64, N_MT + 392 * cc : N_MT + 392 * (cc + 1)],
                )

        def proj_unit(b, pt):
            p0, psz = POS_T[pt]
            ob = wpool.tile([128, C], BF, tag="osb")
            for c0, csz in CCHUNK:
                ps = psum("psPJ")
                for kt in range(6):
                    nc.tensor.matmul(
                        ps[0:psz, 0:csz],
                        aoT[b][kt][:, p0 : p0 + psz],
                        wp[kt][:, c0 : c0 + csz],
                        start=(kt == 0),
                        stop=(kt == 5),
                    )
                nc.vector.tensor_add(
                    ob[0:psz, c0 : c0 + csz], ps[0:psz, 0:csz], pb_bc[0:psz, c0 : c0 + csz]
                )
            nc.sync.dma_start(out_d[b, p0 : p0 + psz, :], ob[0:psz, :])

        def qk_pool_unit(b, m):
            q_unit(b, m)
            if m < 6:
                pool_ct(b, m)

        def qkv_units(b):
            units = []
            for m in range(12):
                units.append(lambda m=m: qk_pool_unit(b, m))
            for pt in range(10):
                units.append(lambda pt=pt: v_unit(b, pt))
            return units

        # ---- schedule ----
        # aoT[b] aliases qkT[b] q-tiles 0-5: by the time a pair's norms write
        # a tile, that pair's s1/s3 score matmuls (its only q readers) are
        # done -- the tile framework serializes the WAR. Saves 14KB/partition
        # and lets proj(b) units 2-9 defer into batch b+1's pair loop as PE
        # fillers (the last batch finally gets filler work).
        load_x(0)
        load_x(1)
        for u in qkv_units(0):
            u()

        prev = None
        for b in range(NB):
            aoT[b] = qkT[b][0:6]
            # fill order matters: proj(b-1) reads qkT[b-1] (same buf parity
            # as qkv(b+1)'s q_unit writes), so proj units must drain first.
            fill = []
            if b > 0:
                fill += [lambda pt=pt, pb=b - 1: proj_unit(pb, pt) for pt in range(2, 10)]
            if b + 1 < NB:
                fill += list(qkv_units(b + 1))
            if b + 2 < NB:
                load_x(b + 2)
            fi = 0
            # last batch has few fillers (proj of b-1 only): ration them
            # across iterations instead of exhausting them in the first two
            ration = 2 if b == NB - 1 else 10**9
            it_taken = [0]

            def take(n):
                nonlocal fi
                for _ in range(n):
                    if fi < len(fill) and it_taken[0] < ration:
                        fill[fi]()
                        fi += 1
                        it_taken[0] += 1

            # cross-pair software pipeline: iteration runs scores(+exps) of
            # pair p while the pv-stages of pair p-1 consume exps computed a
            # full iteration (~17us) earlier -- the scalar exp chain leaves
            # the PE critical path entirely.
            for p2 in range(6):
                st = {}
                it_taken[0] = 0
                if prev is None:
                    pair_scores(b, p2, st)
                    take(2)
                elif prev[1] == 5:
                    # batch boundary: pv-stage of (b-1, 5) woven with
                    # scores(b, 0). No takes until its pv3 norms land --
                    # the proj(b-1) fillers read the s-region they write
                    # (taking one earlier would deadlock the PE queue).
                    pb_, pp_, pst = prev
                    pair_pv1_mm(pb_, pp_, pst)
                    pair_transp(pb_, pp_, pst)
                    pair_pv1_norm_pre(pb_, pp_, pst)
                    pair_scores(b, p2, st)
                    pair_pv2(pb_, pp_, pst)
                    pair_pv1_norm_mul(pb_, pp_, pst)
                    proj_unit(pb_, 0)
                    pair_pv3_mm(pb_, pp_, pst)
                    pair_pv3_norm_pre(pb_, pp_, pst)
                    pair_pv3_norm_mul(pb_, pp_, pst)
                    proj_unit(pb_, 1)
                    take(4)
                elif fi < len(fill) and b < NB - 1:
                    pb_, pp_, pst = prev
                    pair_scores(b, p2, st)
                    pair_pv1_mm(pb_, pp_, pst)
                    pair_transp(pb_, pp_, pst)
                    pair_pv1_norm_pre(pb_, pp_, pst)
                    take(1)
                    pair_pv2(pb_, pp_, pst)
                    pair_pv1_norm_mul(pb_, pp_, pst)
                    take(1)
                    pair_pv3_mm(pb_, pp_, pst)
                    pair_pv3_norm_pre(pb_, pp_, pst)
                    take(1)
                    pair_pv3_norm_mul(pb_, pp_, pst)
                    take(2)
                else:
                    # filler-starved (late b3): weave scores between pv2 and
                    # pv3 so the av vector chain hides under the score
                    # matmuls (same shape as the batch-boundary branch)
                    pb_, pp_, pst = prev
                    pair_pv1_mm(pb_, pp_, pst)
                    pair_transp(pb_, pp_, pst)
                    pair_pv1_norm_pre(pb_, pp_, pst)
                    pair_pv2(pb_, pp_, pst)
                    pair_pv1_norm_mul(pb_, pp_, pst)
                    pair_scores(b, p2, st)
                    pair_pv3_mm(pb_, pp_, pst)
                    pair_pv3_norm_pre(pb_, pp_, pst)
                    take(2)
                    pair_pv3_norm_mul(pb_, pp_, pst)
                prev = (b, p2, st)
            it_taken[0] = -(10**9)  # drain remaining fillers unrationed
            take(len(fill))

        # drain: pv-stage of the final pair + last batch's proj
        pb_, pp_, pst = prev
        pair_pv1_mm(pb_, pp_, pst)
        pair_transp(pb_, pp_, pst)
        pair_pv1_norm_pre(pb_, pp_, pst)
        pair_pv2(pb_, pp_, pst)
        pair_pv1_norm_mul(pb_, pp_, pst)
        proj_unit(pb_, 0)
        proj_unit(pb_, 1)
        pair_pv3_mm(pb_, pp_, pst)
        pair_pv3_norm_pre(pb_, pp_, pst)
        pair_pv3_norm_mul(pb_, pp_, pst)
        for pt in range(2, 10):
            proj_unit(pb_, pt)

    nc.compile()
    return nc


def tc_ctx(nc):
    from contextlib import contextmanager

    @contextmanager
    def ctx():
        with tile.TileContext(nc) as tc, nc.allow_low_precision(reason="attn bf16"):
            with (
                tc.tile_pool(name="const", bufs=1) as cpool,
                tc.tile_pool(name="work", bufs=2) as wpool,
                tc.tile_pool(name="hold", bufs=1) as hpool,
                tc.tile_pool(name="psum", bufs=8, space="PSUM") as ppool,
            ):
                yield tc, cpool, wpool, hpool, ppool

    return ctx()


_PROGRAM = None


def _get_program():
    global _PROGRAM
    if _PROGRAM is None:
        _PROGRAM = build_program()
    return _PROGRAM


def _prep_inputs(x, qkv_w, qkv_b, proj_w, proj_b):
    bf = ml_dtypes.bfloat16
    x = np.asarray(x, dtype=np.float32)
    xT = np.ascontiguousarray(x.transpose(0, 2, 1)).astype(bf)  # [B, C, N]
    wqkT = np.ascontiguousarray(np.asarray(qkv_w, dtype=np.float32).T).astype(bf)
    wpjT = np.ascontiguousarray(np.asarray(proj_w, dtype=np.float32).T).astype(bf)
    qb = np.asarray(qkv_b, dtype=np.float32)
    vbb = np.broadcast_to(qb[2 * 768 :].astype(bf), (128, 768)).copy()
    bqkp = np.ascontiguousarray(qb[: 2 * 768].reshape(12, 128).T).astype(np.float32)
    pbb = np.broadcast_to(
        np.asarray(proj_b, dtype=np.float32).astype(bf), (128, 768)
    ).copy()
    in_maps = []
    for c in range(N_CORES):
        in_maps.append(
            {
                "xT": np.ascontiguousarray(xT[c * NB : (c + 1) * NB]),
                "wqkT": wqkT,
                "wpjT": wpjT,
                "vbb": vbb,
                "bqkp": bqkp,
                "pbb": pbb,
            }
        )
    return in_maps


def kernel(x, qkv_w, qkv_b, proj_w, proj_b, t_h=14, t_w=14, s_h=28, s_w=28, **kw):
    nc = _get_program()
    in_maps = _prep_inputs(x, qkv_w, qkv_b, proj_w, proj_b)
    res = bass_utils.run_bass_kernel_spmd(nc, in_maps, core_ids=list(range(N_CORES)))
    out = np.concatenate([res.results[c]["out"] for c in range(N_CORES)], axis=0)
    return np.asarray(out, dtype=np.float32)


if __name__ == "__main__":
    build_program()
    print("program built OK")



# revision 76
# speedup vs baseline: 1.0100x; 1.0003x over previous
"""AgentAttention Trainium2 kernel: 8-core data-parallel over batch.

v3: cross-PAIR software pipeline on top of v2's cross-batch fillers.
Iteration i runs scores+exps of head-pair p on PE/scalar while the pv
stages of pair p-1 consume exps computed a full iteration (~17us) earlier,
taking the scalar exp chain off the PE critical path (v2 stalled ~124
times/kernel on exp semaphores). Norm chains are split into pre (copy sumexp
row from psum -> recip -> gpsimd broadcast) and mul phases so the in-order
vector queue never head-of-line blocks on gpsimd. aoT aliases the dead q
tiles of qkT (the tile framework serializes the WAR), freeing 14KB/partition
and letting proj(b) units 2-9 defer into batch b+1 as PE fillers -- the
last batch (which has no next-batch qkv work) rations them across its
iterations. v3.1: bias-table DMAs issued before the big weight loads
(first q-evac stalled 5.8us on bqkp), bf16 output DMA with host upcast
(halves 15.6MB of out traffic; +0.2% quantization, rel err 3.5e-3 ->
4.5e-3 vs the 2e-2 gate), av ones-column memset once per rotation slot.
v3.2: filler-starved iterations weave scores between pv2 and pv3 so the
av vector chain hides under score matmuls; wq DMA in q/k/v column thirds.
v3.3: ALL of b3's steady iterations use the weave order (fillers taken at
the pv3-broadcast point); drain front-loads both mt-region proj units
before the pv3 chain (norm muls stay ahead of later proj units -- moving
them behind deadlock-stalls the vector queue). Splitting pair_scores to
interleave pv1 between s1 and s23, or issuing e1 exps before the s2/s3
matmuls, each cost +4-5us: keep all score matmuls then all exps, in claim
order. 605.7us baseline -> 548.0us.

Hard-won negative results (do not redo):
  - gpsimd cannot read PSUM (walrus rejects; custom-DVE recip from psum
    returns garbage on HW even though CoreSim passes).
  - gpsimd tensor_add for the pooling starves the norm broadcasts via
    library swaps: +140us.
  - dma_start on the scalar queue head-of-line blocks exps: +120us.
  - 2-bank psum claims with merged exps/evacs halve scalar ops but the
    4-deep rotation + coarser evac latency cost +56us net.
  - fp8 DoubleRow for qkv/proj GEMMs: e4m3's ~3% RMS error vs the 2e-2
    max-abs/absmax gate (= 3.5e-3 abs diff) fails at near-absmax outputs.
  - matmul PSUM dst at a sub-bank column offset (256B tried) raises a
    runtime exec fault; transposes tolerate sub-bank offsets, regular
    matmuls do not -- dsts must start at a bank boundary.
  - computing s2 directly transposed ([keys, agents] via a block-diagonal
    zero-padded agT) kills the 10 PE transposes + eT copies per pair but
    needs 10 one-bank psum claims and 10 small exps per pair: +83us net
    (claim-rotation stalls inside the scores stage dominate).
  - interleaving the tiny bqkp DMA between the wq half-loads (to exploit
    issue-order-cumulative completion waits) plus front-loading drain proj
    units: +100us -- do not reorder the const-DMA prologue.

Remaining headroom map (measured on the 548us config; wall 554us that
rep): tensor busy 499.7us / vector 377.6 / scalar 238.2. PE gaps 49.9us =
14.5 startup (first ldweights waits the ENTIRE const-DMA group no matter
how the loads are split -- needs per-tile DMA completion granularity) +
10.6 tail (b3 proj chain + out-DMA drain) + 24.8 steady micro-gaps
(~190 claim-rotation/exp-latency stalls of ~130ns). On top, ~60us of
p-state ramp penalty is embedded in the busy time (ideal full-clock PE
~437us). All three point to the same structural fix: depth-2 pv
pipelining (iteration i = scores(p), pv1/transp/pv2(p-1), pv3(p-2)) so
every cross-engine dependency gets two iterations of slack; needs e-tile
bufs for 3 pairs in flight (~+8KB SBUF, available).

Layouts (per core, 4 batches):
  xT      [4, 768, 1176] bf16  (c-major x)
  qkT     c-major q,k: 12 sbuf tiles [128, 1176] (tiles 0-5 = q, 6-11 = k);
          q tiles are overwritten in place by attention output (aoT alias)
  v_ext   pos-major v with per-head ones column (col 64): 10 tiles [128, 12*65]
  agT     pooled agent tokens (sums over 4x4 blocks), c-major [128, 49] x6
  aoT     = qkT[0:6] (bf16) -> proj -> out
Matmuls bf16, fp32 psum (uniform pool of 8 one-bank tiles [128,512]).
Softmax scale folded into ACT exp scale (0.125 stage1; 0.125/16 stages 2/3
-- agent tokens are pooled SUMS). qk bias via per-partition activation bias.
"""

import sys

sys.path.insert(0, "/opt/trn_rl_repo")

import numpy as np
import ml_dtypes

import concourse.bass as bass
import concourse.mybir as mybir
import concourse.tile as tile
from concourse import bacc, bass_utils
from concourse.masks import make_identity

BF = mybir.dt.bfloat16
F32 = mybir.dt.float32
AF = mybir.ActivationFunctionType

N_CORES = 8
B, N, C = 32, 1176, 768
NB = B // N_CORES
H, HD = 12, 64
N_MT, N_S = 392, 784
A = 49
SCALE1 = 0.125
SCALE23 = 0.125 / 16.0

POS_T = [(pt * 128, min(128, N - pt * 128)) for pt in range(10)]
KEY1_T = [(0, 128), (128, 128), (256, 128), (384, 8)]
NCHUNK = [(0, 392), (392, 392), (784, 392)]
CCHUNK = [(0, 512), (512, 256)]
TSP = 116  # transpose chunk col spacing (>=113, even)


def build_program():
    nc = bacc.Bacc("TRN2", debug=False, num_devices=N_CORES)

    xT_d = nc.dram_tensor("xT", [NB, C, N], BF, kind="ExternalInput").ap()
    wqkT_d = nc.dram_tensor("wqkT", [C, 3 * C], BF, kind="ExternalInput").ap()
    wpjT_d = nc.dram_tensor("wpjT", [C, C], BF, kind="ExternalInput").ap()
    vbb_d = nc.dram_tensor("vbb", [128, C], BF, kind="ExternalInput").ap()
    bqkp_d = nc.dram_tensor("bqkp", [128, 12], F32, kind="ExternalInput").ap()
    pbb_d = nc.dram_tensor("pbb", [128, C], BF, kind="ExternalInput").ap()
    # bf16 output (host upcasts): halves 15.6MB of out-DMA traffic and the
    # end-of-kernel drain; adds ~0.2% quantization, well inside the gate
    out_d = nc.dram_tensor("out", [NB, N, C], BF, kind="ExternalOutput").ap()

    with tc_ctx(nc) as (tc, cpool, wpool, hpool, ppool):
        # ---- one-time constants/weights ----
        # q/k and v weight columns live in SEPARATE tiles: dependency
        # tracking is tile-granular, so the first ldweights on a combined
        # tile waits for its LAST DMA (the v third, ~10us in). Split tiles
        # let the opening q_unit start ~1us after its own contiguous load.
        wq = [
            cpool.tile([128, 2 * C], BF, tag=f"wq{i}", name=f"wq{i}") for i in range(6)
        ]
        wv = [cpool.tile([128, C], BF, tag=f"wv{i}", name=f"wv{i}") for i in range(6)]
        wp = [cpool.tile([128, C], BF, tag=f"wp{i}", name=f"wp{i}") for i in range(6)]
        # tiny bias tables FIRST: the first q-evac needs bqkp and stalled
        # 5.8us queued behind the big weight loads
        bqkp = cpool.tile([128, 12], F32, tag="bqkp")
        nc.sync.dma_start(bqkp[:], bqkp_d[:])
        vb_bc = cpool.tile([128, C], BF, tag="vb_bc")
        nc.sync.dma_start(vb_bc[:], vbb_d[:])
        pb_bc = cpool.tile([128, C], BF, tag="pb_bc")
        nc.sync.dma_start(pb_bc[:], pbb_d[:])
        for i in range(6):
            nc.sync.dma_start(wq[i][:], wqkT_d[128 * i : 128 * (i + 1), 0 : 2 * C])
        for i in range(6):
            nc.sync.dma_start(wv[i][:], wqkT_d[128 * i : 128 * (i + 1), 2 * C : 3 * C])
        for i in range(6):
            nc.sync.dma_start(wp[i][:], wpjT_d[128 * i : 128 * (i + 1), :])
        ident = cpool.tile([128, 128], BF, tag="ident")
        make_identity(nc, ident[:])

        pv2_calls = [0]

        # per-batch tile handles (rotated via tags, bufs=2)
        xT = {}
        qkT = {}
        v_ext = {}
        agT = {}
        aoT = {}

        def psum(name):
            return ppool.tile([128, 512], F32, tag="P", name=name, bufs=8)

        def load_x(b):
            xT[b] = [
                hpool.tile([128, N], BF, tag=f"xT{i}", name=f"xT{i}", bufs=2)
                for i in range(6)
            ]
            eng = nc.scalar if b == 0 else nc.sync
            for i in range(6):
                eng.dma_start(xT[b][i][:], xT_d[b, 128 * i : 128 * (i + 1), :])

        def q_unit(b, m):
            # qkT[m] c-major [128, 1176] for q (m<6) / k (m>=6) rows
            if m == 0:
                qkT[b] = [None] * 12
            t = hpool.tile([128, N], BF, tag=f"qkT{m}", name=f"qkT{m}", bufs=2)
            qkT[b][m] = t
            for n0, nsz in NCHUNK:
                ps = psum("psQ")
                for kt in range(6):
                    nc.tensor.matmul(
                        ps[:, 0:nsz],
                        wq[kt][:, 128 * m : 128 * (m + 1)],
                        xT[b][kt][:, n0 : n0 + nsz],
                        start=(kt == 0),
                        stop=(kt == 5),
                    )
                # evac on scalar engine (gpsimd cannot read PSUM)
                nc.scalar.activation(
                    t[:, n0 : n0 + nsz],
                    ps[:, 0:nsz],
                    AF.Identity,
                    bias=bqkp[:, m : m + 1],
                )

        def v_unit(b, pt):
            # pos-major v_ext [psz, 12*65] with ones col at 64 of each head.
            # both c-chunks in one 2-bank claim -> ONE merged evac add
            p0, psz = POS_T[pt]
            if pt == 0:
                v_ext[b] = [None] * 10
            vt = hpool.tile([128, H * 65], BF, tag=f"vx{pt}", name=f"vx{pt}", bufs=2)
            v_ext[b][pt] = vt
            if b < 2:
                # two rotation slots; evac only writes the 64 v columns, so
                # ones persist across later batches
                nc.vector.memset(
                    vt[:].rearrange("p (h e) -> p h e", e=65)[:, :, 64:65], 1.0
                )
            for ci, (c0, csz) in enumerate(CCHUNK):
                ps = psum("psV")
                for kt in range(6):
                    nc.tensor.matmul(
                        ps[0:psz, 0:csz],
                        xT[b][kt][:, p0 : p0 + psz],
                        wv[kt][:, c0 : c0 + csz],
                        start=(kt == 0),
                        stop=(kt == 5),
                    )
                nh = csz // 64
                h0 = c0 // 64
                nc.vector.tensor_add(
                    vt[0:psz].rearrange("p (h e) -> p h e", e=65)[
                        :, h0 : h0 + nh, 0:64
                    ],
                    ps[0:psz, 0:csz].rearrange("p (h d) -> p h d", d=64),
                    vb_bc[0:psz, c0 : c0 + csz].rearrange("p (h d) -> p h d", d=64),
                )

        def pool_ct(b, ct):
            # sum 4x4 blocks of q_s -> agT (c-major). On VECTOR: gpsimd must
            # stay broadcast-only (lib swaps + in-order blocking starve the
            # norm-chain broadcasts otherwise)
            if ct == 0:
                agT[b] = []
            t1 = wpool.tile([128, 196], F32, tag="t1", bufs=1)
            qs = qkT[b][ct][:, N_MT:N]  # [128, 784], idx = i*28 + aj*4 + dj
            q4 = qs.rearrange("p (x dj) -> p x dj", dj=4)
            nc.vector.tensor_add(t1[:, 0:196], q4[:, :, 0:1], q4[:, :, 1:2])
            nc.vector.tensor_add(t1[:, 0:196], t1[:, 0:196], q4[:, :, 2:3])
            nc.vector.tensor_add(t1[:, 0:196], t1[:, 0:196], q4[:, :, 3:4])
            ag = hpool.tile([128, A], BF, tag=f"ag{ct}", name=f"ag{ct}", bufs=2)
            agT[b].append(ag)
            # t1 idx = 28*ai + 7*di + aj -> view (ai, aj, di)
            t4 = t1[:, 0:196].rearrange("p (ai di aj) -> p ai aj di", ai=7, di=4)
            t2 = wpool.tile([128, A], F32, tag="t2")
            nc.vector.tensor_add(t2[:, 0:A], t4[:, :, :, 0:1], t4[:, :, :, 1:2])
            nc.vector.tensor_add(t2[:, 0:A], t2[:, 0:A], t4[:, :, :, 2:3])
            nc.vector.tensor_add(ag[:, 0:A], t2[:, 0:A], t4[:, :, :, 3:4])

        def norm_pre(pv, c0):
            # recip of the psum sumexp row, broadcast to 64 partitions.
            # (custom-DVE recip reading PSUM directly returns garbage on HW;
            # stage the sumexp row through SBUF first.) Split from the mul so
            # the in-order vector queue never head-of-line blocks on gpsimd.
            se = wpool.tile([1, 392], F32, tag="se", bufs=2)
            nc.vector.tensor_copy(se[:, 0:392], pv[64:65, c0 : c0 + 392])
            rc = wpool.tile([1, 392], F32, tag="rc", bufs=2)
            nc.vector.reciprocal_approx_fast(out=rc[:, 0:392], in_=se[:, 0:392])
            bc = wpool.tile([64, 392], F32, tag="bc", bufs=6)
            nc.gpsimd.partition_broadcast(bc[:], rc[0:1, 0:392])
            return bc

        def norm_mul(pv, c0, bc, dst):
            nc.vector.tensor_mul(dst, pv[0:64, c0 : c0 + 392], bc[:])

        # ---- attention for one head pair, split into schedulable chunks ----
        def pair_scores_s1(b, p2, st):
            qt = p2
            # stage 1 scores first: [keys, queries] per head over 4 key chunks.
            # Claim order matches exp (= psum evacuation) order so the 8-bank
            # rotation never waits, and e1 (pv1's dep) is computed earliest.
            st["s1"] = []
            for hp in range(2):
                qo = 64 * hp
                chunks = []
                st["s1"].append(chunks)
                for k0, ksz in KEY1_T:
                    ps = psum("psS1")
                    chunks.append(ps)
                    nc.tensor.matmul(
                        ps[0:ksz, 0:392],
                        qkT[b][6 + qt][qo : qo + 64, k0 : k0 + ksz],
                        qkT[b][qt][qo : qo + 64, 0:N_MT],
                        start=True,
                        stop=True,
                    )
        def pair_scores_s23(b, p2, st):
            qt = p2
            # stage 2 scores: [49x2 packed, keys] over 3 chunks
            st["s2"] = []
            for n0, nsz in NCHUNK:
                ps = psum("psS2")
                st["s2"].append(ps)
                for hp in range(2):
                    qo = 64 * hp
                    nc.tensor.matmul(
                        ps[qo : qo + 49, 0:nsz],
                        agT[b][qt][qo : qo + 64, 0:A],
                        qkT[b][6 + qt][qo : qo + 64, n0 : n0 + nsz],
                        start=True,
                        stop=True,
                    )
            # stage 3 scores: [49x2 packed (agents), queries] over 2 chunks
            st["s3"] = []
            for cc in range(2):
                ps = psum("psS3")
                st["s3"].append(ps)
                for hp in range(2):
                    qo = 64 * hp
                    nc.tensor.matmul(
                        ps[qo : qo + 49, 0:392],
                        agT[b][qt][qo : qo + 64, 0:A],
                        qkT[b][qt][qo : qo + 64, N_MT + 392 * cc : N_MT + 392 * (cc + 1)],
                        start=True,
                        stop=True,
                    )
            # exps (scalar engine) in the same order as the score claims.
            # bufs sized for TWO pairs in flight (cross-pair pipeline)
            st["e1"] = []
            for hp in range(2):
                e1s = []
                st["e1"].append(e1s)
                for j, (k0, ksz) in enumerate(KEY1_T):
                    e1 = wpool.tile([128, 392], BF, tag="e1", name="e1", bufs=16)
                    e1s.append(e1)
                    nc.scalar.activation(
                        e1[0:ksz, 0:392],
                        st["s1"][hp][j][0:ksz, 0:392],
                        AF.Exp,
                        scale=SCALE1,
                    )
            e2 = wpool.tile([128, N], BF, tag="e2", bufs=2)
            st["e2"] = e2
            for j, (n0, nsz) in enumerate(NCHUNK):
                nc.scalar.activation(
                    e2[0:113, n0 : n0 + nsz],
                    st["s2"][j][0:113, 0:nsz],
                    AF.Exp,
                    scale=SCALE23,
                )
            st["e3"] = []
            for cc in range(2):
                e3 = wpool.tile([128, 392], BF, tag="e3", name="e3", bufs=4)
                st["e3"].append(e3)
                nc.scalar.activation(
                    e3[0:113, 0:392], st["s3"][cc][0:113, 0:392], AF.Exp, scale=SCALE23
                )

        def pair_scores(b, p2, st):
            pair_scores_s1(b, p2, st)
            pair_scores_s23(b, p2, st)

        def pair_pv1_mm(b, p2, st):
            st["pv1"] = []
            for hp in range(2):
                pv = psum("psPV1")
                st["pv1"].append(pv)
                for j, (k0, ksz) in enumerate(KEY1_T):
                    nc.tensor.matmul(
                        pv[0:65, 0:392],
                        v_ext[b][j][0:ksz, 65 * (2 * p2 + hp) : 65 * (2 * p2 + hp) + 65],
                        st["e1"][hp][j][0:ksz, 0:392],
                        start=(j == 0),
                        stop=(j == 3),
                    )

        def pair_pv1_norm_pre(b, p2, st):
            st["bc1"] = [norm_pre(st["pv1"][hp], 0) for hp in range(2)]

        def pair_pv1_norm_mul(b, p2, st):
            qt = p2
            for hp in range(2):
                qo = 64 * hp
                norm_mul(
                    st["pv1"][hp], 0, st["bc1"][hp],
                    aoT[b][qt][qo : qo + 64, 0:N_MT],
                )

        def pair_transp(b, p2, st):
            # [113, keys] -> [keys, 113] in 10 chunks, via identity matmul
            st["eT"] = []
            for half in range(2):
                trp = ppool.tile([128, 5 * TSP], BF, tag="P", name="psTr", bufs=8)
                for kk in range(5):
                    kt = 5 * half + kk
                    p0, psz = POS_T[kt]
                    nc.tensor.transpose(
                        trp[0:psz, TSP * kk : TSP * kk + 113],
                        st["e2"][0:113, p0 : p0 + psz],
                        ident[0:113, 0:113],
                    )
                eT = wpool.tile([128, 5 * TSP], BF, tag="e2T", bufs=4)
                st["eT"].append(eT)
                # evac on vector: scalar is exp-saturated in the pair slot
                nc.vector.tensor_copy(eT[:, 0 : 5 * TSP], trp[:, 0 : 5 * TSP])

        def pair_pv2(b, p2, st, do_memset=False):
            # both heads per matmul: lhsT = full transposed tile (garbage rows
            # 49:63 only pollute unused output rows), rhs = 129-wide v_ext
            # slice [v_h0 | ones | v_h1]; the ones col yields both heads'
            # sumexp at out col 64. 10 matmuls instead of 20.
            do_memset = do_memset or pv2_calls[0] < 2
            pv2_calls[0] += 1
            pv2 = psum("psPV2")
            for kt, (p0, psz) in enumerate(POS_T):
                eT = st["eT"][kt // 5]
                cof = TSP * (kt % 5)
                nc.tensor.matmul(
                    pv2[0:113, 0:129],
                    eT[0:psz, cof : cof + 113],
                    v_ext[b][kt][0:psz, 130 * p2 : 130 * p2 + 129],
                    start=(kt == 0),
                    stop=(kt == 9),
                )
            av = wpool.tile([128, 65], BF, tag="avx", bufs=2)
            st["av"] = av
            if do_memset:
                # two rotation slots; ts_muls only write cols 0:64, so the
                # ones column persists across later pairs
                nc.vector.memset(av[0:113, 64:65], 1.0)
            avr = wpool.tile([128, 1], F32, tag="avr", bufs=2)
            nc.vector.reciprocal(avr[0:113, 0:1], pv2[0:113, 64:65])
            nc.vector.tensor_scalar_mul(av[0:49, 0:64], pv2[0:49, 0:64], avr[0:49, 0:1])
            nc.vector.tensor_scalar_mul(
                av[64:113, 0:64], pv2[64:113, 65:129], avr[64:113, 0:1]
            )

        def pair_pv3_mm(b, p2, st):
            st["pv3"] = []
            for hp in range(2):
                for cc in range(2):
                    pv = psum("psPV3")
                    st["pv3"].append(pv)
                    nc.tensor.matmul(
                        pv[0:65, 0:392],
                        st["av"][64 * hp : 64 * hp + 49, 0:65],
                        st["e3"][cc][64 * hp : 64 * hp + 49, 0:392],
                        start=True,
                        stop=True,
                    )

        def pair_pv3_norm_pre(b, p2, st):
            st["bc3"] = [norm_pre(pv, 0) for pv in st["pv3"]]

        def pair_pv3_norm_mul(b, p2, st):
            qt = p2
            for i, pv in enumerate(st["pv3"]):
                hp, cc = divmod(i, 2)
                norm_mul(
                    pv,
                    0,
                    st["bc3"][i],
                    aoT[b][qt][64 * hp : 64 * hp + 64, N_MT + 392 * cc : N_MT + 392 * (cc + 1)],
                )

        def proj_unit(b, pt):
            p0, psz = POS_T[pt]
            ob = wpool.tile([128, C], BF, tag="osb")
            for c0, csz in CCHUNK:
                ps = psum("psPJ")
                for kt in range(6):
                    nc.tensor.matmul(
                        ps[0:psz, 0:csz],
                        aoT[b][kt][:, p0 : p0 + psz],
                        wp[kt][:, c0 : c0 + csz],
                        start=(kt == 0),
                        stop=(kt == 5),
                    )
                nc.vector.tensor_add(
                    ob[0:psz, c0 : c0 + csz], ps[0:psz, 0:csz], pb_bc[0:psz, c0 : c0 + csz]
                )
            nc.sync.dma_start(out_d[b, p0 : p0 + psz, :], ob[0:psz, :])

        def qk_pool_unit(b, m):
            q_unit(b, m)
            if m < 6:
                pool_ct(b, m)

        def qkv_units(b):
            units = []
            for m in range(12):
                units.append(lambda m=m: qk_pool_unit(b, m))
            for pt in range(10):
                units.append(lambda pt=pt: v_unit(b, pt))
            return units

        # ---- schedule ----
        # aoT[b] aliases qkT[b] q-tiles 0-5: by the time a pair's norms write
        # a tile, that pair's s1/s3 score matmuls (its only q readers) are
        # done -- the tile framework serializes the WAR. Saves 14KB/partition
        # and lets proj(b) units 2-9 defer into batch b+1's pair loop as PE
        # fillers (the last batch finally gets filler work).
        load_x(0)
        load_x(1)
        for u in qkv_units(0):
            u()

        prev = None
        for b in range(NB):
            aoT[b] = qkT[b][0:6]
            # fill order matters: proj(b-1) reads qkT[b-1] (same buf parity
            # as qkv(b+1)'s q_unit writes), so proj units must drain first.
            fill = []
            if b > 0:
                fill += [lambda pt=pt, pb=b - 1: proj_unit(pb, pt) for pt in range(2, 10)]
            if b + 1 < NB:
                fill += list(qkv_units(b + 1))
            if b + 2 < NB:
                load_x(b + 2)
            fi = 0
            # last batch has few fillers (proj of b-1 only): ration them
            # across iterations instead of exhausting them in the first two
            ration = 2 if b == NB - 1 else 10**9
            it_taken = [0]

            def take(n):
                nonlocal fi
                for _ in range(n):
                    if fi < len(fill) and it_taken[0] < ration:
                        fill[fi]()
                        fi += 1
                        it_taken[0] += 1

            # cross-pair software pipeline: iteration runs scores(+exps) of
            # pair p while the pv-stages of pair p-1 consume exps computed a
            # full iteration (~17us) earlier -- the scalar exp chain leaves
            # the PE critical path entirely.
            for p2 in range(6):
                st = {}
                it_taken[0] = 0
                if prev is None:
                    pair_scores(b, p2, st)
                    take(2)
                elif prev[1] == 5:
                    # batch boundary: pv-stage of (b-1, 5) woven with
                    # scores(b, 0). No takes until its pv3 norms land --
                    # the proj(b-1) fillers read the s-region they write
                    # (taking one earlier would deadlock the PE queue).
                    pb_, pp_, pst = prev
                    pair_pv1_mm(pb_, pp_, pst)
                    pair_transp(pb_, pp_, pst)
                    pair_pv1_norm_pre(pb_, pp_, pst)
                    pair_scores(b, p2, st)
                    pair_pv2(pb_, pp_, pst)
                    pair_pv1_norm_mul(pb_, pp_, pst)
                    proj_unit(pb_, 0)
                    pair_pv3_mm(pb_, pp_, pst)
                    pair_pv3_norm_pre(pb_, pp_, pst)
                    pair_pv3_norm_mul(pb_, pp_, pst)
                    proj_unit(pb_, 1)
                    take(4)
                elif fi < len(fill) and b < NB - 1:
                    pb_, pp_, pst = prev
                    pair_scores(b, p2, st)
                    pair_pv1_mm(pb_, pp_, pst)
                    pair_transp(pb_, pp_, pst)
                    pair_pv1_norm_pre(pb_, pp_, pst)
                    take(1)
                    pair_pv2(pb_, pp_, pst)
                    pair_pv1_norm_mul(pb_, pp_, pst)
                    take(1)
                    pair_pv3_mm(pb_, pp_, pst)
                    pair_pv3_norm_pre(pb_, pp_, pst)
                    take(1)
                    pair_pv3_norm_mul(pb_, pp_, pst)
                    take(2)
                else:
                    # filler-starved (late b3): weave scores between pv2 and
                    # pv3 so the av vector chain hides under the score
                    # matmuls (same shape as the batch-boundary branch)
                    pb_, pp_, pst = prev
                    pair_pv1_mm(pb_, pp_, pst)
                    pair_transp(pb_, pp_, pst)
                    pair_pv1_norm_pre(pb_, pp_, pst)
                    pair_pv2(pb_, pp_, pst)
                    pair_pv1_norm_mul(pb_, pp_, pst)
                    pair_scores(b, p2, st)
                    pair_pv3_mm(pb_, pp_, pst)
                    pair_pv3_norm_pre(pb_, pp_, pst)
                    take(2)
                    pair_pv3_norm_mul(pb_, pp_, pst)
                prev = (b, p2, st)
            it_taken[0] = -(10**9)  # drain remaining fillers unrationed
            take(len(fill))

        # drain: pv-stage of the final pair + last batch's proj
        pb_, pp_, pst = prev
        pair_pv1_mm(pb_, pp_, pst)
        pair_transp(pb_, pp_, pst)
        pair_pv1_norm_pre(pb_, pp_, pst)
        pair_pv2(pb_, pp_, pst)
        pair_pv1_norm_mul(pb_, pp_, pst)
        proj_unit(pb_, 0)
        proj_unit(pb_, 1)
        pair_pv3_mm(pb_, pp_, pst)
        pair_pv3_norm_pre(pb_, pp_, pst)
        pair_pv3_norm_mul(pb_, pp_, pst)
        for pt in range(2, 10):
            proj_unit(pb_, pt)

    nc.compile()
    return nc


def tc_ctx(nc):
    from contextlib import contextmanager

    @contextmanager
    def ctx():
        with tile.TileContext(nc) as tc, nc.allow_low_precision(reason="attn bf16"):
            with (
                tc.tile_pool(name="const", bufs=1) as cpool,
                tc.tile_pool(name="work", bufs=2) as wpool,
                tc.tile_pool(name="hold", bufs=1) as hpool,
                tc.tile_pool(name="psum", bufs=8, space="PSUM") as ppool,
            ):
                yield tc, cpool, wpool, hpool, ppool

    return ctx()


_PROGRAM = None


def _get_program():
    global _PROGRAM
    if _PROGRAM is None:
        _PROGRAM = build_program()
    return _PROGRAM


def _prep_inputs(x, qkv_w, qkv_b, proj_w, proj_b):
    bf = ml_dtypes.bfloat16
    x = np.asarray(x, dtype=np.float32)
    xT = np.ascontiguousarray(x.transpose(0, 2, 1)).astype(bf)  # [B, C, N]
    wqkT = np.ascontiguousarray(np.asarray(qkv_w, dtype=np.float32).T).astype(bf)
    wpjT = np.ascontiguousarray(np.asarray(proj_w, dtype=np.float32).T).astype(bf)
    qb = np.asarray(qkv_b, dtype=np.float32)
    vbb = np.broadcast_to(qb[2 * 768 :].astype(bf), (128, 768)).copy()
    bqkp = np.ascontiguousarray(qb[: 2 * 768].reshape(12, 128).T).astype(np.float32)
    pbb = np.broadcast_to(
        np.asarray(proj_b, dtype=np.float32).astype(bf), (128, 768)
    ).copy()
    in_maps = []
    for c in range(N_CORES):
        in_maps.append(
            {
                "xT": np.ascontiguousarray(xT[c * NB : (c + 1) * NB]),
                "wqkT": wqkT,
                "wpjT": wpjT,
                "vbb": vbb,
                "bqkp": bqkp,
                "pbb": pbb,
            }
        )
    return in_maps


def kernel(x, qkv_w, qkv_b, proj_w, proj_b, t_h=14, t_w=14, s_h=28, s_w=28, **kw):
    nc = _get_program()
    in_maps = _prep_inputs(x, qkv_w, qkv_b, proj_w, proj_b)
    res = bass_utils.run_bass_kernel_spmd(nc, in_maps, core_ids=list(range(N_CORES)))
    out = np.concatenate([res.results[c]["out"] for c in range(N_CORES)], axis=0)
    return np.asarray(out, dtype=np.float32)


if __name__ == "__main__":
    build_program()
    print("program built OK")



# revision 78
# speedup vs baseline: 1.0111x; 1.0011x over previous
"""AgentAttention Trainium2 kernel: 8-core data-parallel over batch.

v3: cross-PAIR software pipeline on top of v2's cross-batch fillers.
Iteration i runs scores+exps of head-pair p on PE/scalar while the pv
stages of pair p-1 consume exps computed a full iteration (~17us) earlier,
taking the scalar exp chain off the PE critical path (v2 stalled ~124
times/kernel on exp semaphores). Norm chains are split into pre (copy sumexp
row from psum -> recip -> gpsimd broadcast) and mul phases so the in-order
vector queue never head-of-line blocks on gpsimd. aoT aliases the dead q
tiles of qkT (the tile framework serializes the WAR), freeing 14KB/partition
and letting proj(b) units 2-9 defer into batch b+1 as PE fillers -- the
last batch (which has no next-batch qkv work) rations them across its
iterations. v3.1: bias-table DMAs issued before the big weight loads
(first q-evac stalled 5.8us on bqkp), bf16 output DMA with host upcast
(halves 15.6MB of out traffic; +0.2% quantization, rel err 3.5e-3 ->
4.5e-3 vs the 2e-2 gate), av ones-column memset once per rotation slot.
v3.2: filler-starved iterations weave scores between pv2 and pv3 so the
av vector chain hides under score matmuls; wq DMA in q/k/v column thirds.
v3.3: ALL of b3's steady iterations use the weave order (fillers taken at
the pv3-broadcast point); drain front-loads both mt-region proj units
before the pv3 chain (norm muls stay ahead of later proj units -- moving
them behind deadlock-stalls the vector queue). Splitting pair_scores to
interleave pv1 between s1 and s23, or issuing e1 exps before the s2/s3
matmuls, each cost +4-5us: keep all score matmuls then all exps, in claim
order. v3.4: q/k and v weight columns split into separate TILES (dep
tracking is tile-granular -- a combined tile's first ldweights waited for
its last DMA third). 605.7us baseline -> 547.8us.

Hard-won negative results (do not redo):
  - gpsimd cannot read PSUM (walrus rejects; custom-DVE recip from psum
    returns garbage on HW even though CoreSim passes).
  - gpsimd tensor_add for the pooling starves the norm broadcasts via
    library swaps: +140us.
  - dma_start on the scalar queue head-of-line blocks exps: +120us.
  - 2-bank psum claims with merged exps/evacs halve scalar ops but the
    4-deep rotation + coarser evac latency cost +56us net.
  - fp8 DoubleRow for qkv/proj GEMMs: e4m3's ~3% RMS error vs the 2e-2
    max-abs/absmax gate (= 3.5e-3 abs diff) fails at near-absmax outputs.
  - matmul PSUM dst at a sub-bank column offset (256B tried) raises a
    runtime exec fault; transposes tolerate sub-bank offsets, regular
    matmuls do not -- dsts must start at a bank boundary.
  - computing s2 directly transposed ([keys, agents] via a block-diagonal
    zero-padded agT) kills the 10 PE transposes + eT copies per pair but
    needs 10 one-bank psum claims and 10 small exps per pair: +83us net
    (claim-rotation stalls inside the scores stage dominate).
  - interleaving the tiny bqkp DMA between the wq half-loads (to exploit
    issue-order-cumulative completion waits) plus front-loading drain proj
    units: +100us -- do not reorder the const-DMA prologue.

Remaining headroom map (measured on the 548us config; wall 554us that
rep): tensor busy 499.7us / vector 377.6 / scalar 238.2. PE gaps 49.9us =
14.5 startup (mostly remains even after the wq/wv tile split: the queue
completion counter still gates on issue order; would need weights on a
second queue ahead of x, or compute warmup that needs no weights) +
10.6 tail (b3 proj chain + out-DMA drain) + 24.8 steady micro-gaps
(~190 claim-rotation/exp-latency stalls of ~130ns). On top, ~60us of
p-state ramp penalty is embedded in the busy time (ideal full-clock PE
~437us). All three point to the same structural fix: depth-2 pv
pipelining (iteration i = scores(p), pv1/transp/pv2(p-1), pv3(p-2)) so
every cross-engine dependency gets two iterations of slack; needs e-tile
bufs for 3 pairs in flight (~+8KB SBUF, available).

Layouts (per core, 4 batches):
  xT      [4, 768, 1176] bf16  (c-major x)
  qkT     c-major q,k: 12 sbuf tiles [128, 1176] (tiles 0-5 = q, 6-11 = k);
          q tiles are overwritten in place by attention output (aoT alias)
  v_ext   pos-major v with per-head ones column (col 64): 10 tiles [128, 12*65]
  agT     pooled agent tokens (sums over 4x4 blocks), c-major [128, 49] x6
  aoT     = qkT[0:6] (bf16) -> proj -> out
Matmuls bf16, fp32 psum (uniform pool of 8 one-bank tiles [128,512]).
Softmax scale folded into ACT exp scale (0.125 stage1; 0.125/16 stages 2/3
-- agent tokens are pooled SUMS). qk bias via per-partition activation bias.
"""

import sys

sys.path.insert(0, "/opt/trn_rl_repo")

import numpy as np
import ml_dtypes

import concourse.bass as bass
import concourse.mybir as mybir
import concourse.tile as tile
from concourse import bacc, bass_utils
from concourse.masks import make_identity

BF = mybir.dt.bfloat16
F32 = mybir.dt.float32
AF = mybir.ActivationFunctionType

N_CORES = 8
B, N, C = 32, 1176, 768
NB = B // N_CORES
H, HD = 12, 64
N_MT, N_S = 392, 784
A = 49
SCALE1 = 0.125
SCALE23 = 0.125 / 16.0

POS_T = [(pt * 128, min(128, N - pt * 128)) for pt in range(10)]
KEY1_T = [(0, 128), (128, 128), (256, 128), (384, 8)]
NCHUNK = [(0, 392), (392, 392), (784, 392)]
CCHUNK = [(0, 512), (512, 256)]
TSP = 116  # transpose chunk col spacing (>=113, even)


def build_program():
    nc = bacc.Bacc("TRN2", debug=False, num_devices=N_CORES)

    xT_d = nc.dram_tensor("xT", [NB, C, N], BF, kind="ExternalInput").ap()
    wqkT_d = nc.dram_tensor("wqkT", [C, 3 * C], BF, kind="ExternalInput").ap()
    wpjT_d = nc.dram_tensor("wpjT", [C, C], BF, kind="ExternalInput").ap()
    vbb_d = nc.dram_tensor("vbb", [128, C], BF, kind="ExternalInput").ap()
    bqkp_d = nc.dram_tensor("bqkp", [128, 12], F32, kind="ExternalInput").ap()
    pbb_d = nc.dram_tensor("pbb", [128, C], BF, kind="ExternalInput").ap()
    # bf16 output (host upcasts): halves 15.6MB of out-DMA traffic and the
    # end-of-kernel drain; adds ~0.2% quantization, well inside the gate
    out_d = nc.dram_tensor("out", [NB, N, C], BF, kind="ExternalOutput").ap()

    with tc_ctx(nc) as (tc, cpool, wpool, hpool, ppool):
        # ---- one-time constants/weights ----
        # q/k and v weight columns live in SEPARATE tiles: dependency
        # tracking is tile-granular, so the first ldweights on a combined
        # tile waits for its LAST DMA (the v third, ~10us in). Split tiles
        # let the opening q_unit start ~1us after its own contiguous load.
        wq = [
            cpool.tile([128, 2 * C], BF, tag=f"wq{i}", name=f"wq{i}") for i in range(6)
        ]
        wv = [cpool.tile([128, C], BF, tag=f"wv{i}", name=f"wv{i}") for i in range(6)]
        wp = [cpool.tile([128, C], BF, tag=f"wp{i}", name=f"wp{i}") for i in range(6)]
        # tiny bias tables FIRST: the first q-evac needs bqkp and stalled
        # 5.8us queued behind the big weight loads
        bqkp = cpool.tile([128, 12], F32, tag="bqkp")
        nc.sync.dma_start(bqkp[:], bqkp_d[:])
        vb_bc = cpool.tile([128, C], BF, tag="vb_bc")
        nc.sync.dma_start(vb_bc[:], vbb_d[:])
        pb_bc = cpool.tile([128, C], BF, tag="pb_bc")
        nc.sync.dma_start(pb_bc[:], pbb_d[:])
        for i in range(6):
            nc.sync.dma_start(wq[i][:], wqkT_d[128 * i : 128 * (i + 1), 0 : 2 * C])

        def load_wvp():
            # emitted AFTER the q/k units: DMA wait thresholds are taken at
            # emission time, so the opening ldweights only counts the 9 DMAs
            # ahead of it instead of all 21
            for i in range(6):
                nc.sync.dma_start(wv[i][:], wqkT_d[128 * i : 128 * (i + 1), 2 * C : 3 * C])
            for i in range(6):
                nc.sync.dma_start(wp[i][:], wpjT_d[128 * i : 128 * (i + 1), :])

        ident = cpool.tile([128, 128], BF, tag="ident")
        make_identity(nc, ident[:])

        pv2_calls = [0]

        # per-batch tile handles (rotated via tags, bufs=2)
        xT = {}
        qkT = {}
        v_ext = {}
        agT = {}
        aoT = {}

        def psum(name):
            return ppool.tile([128, 512], F32, tag="P", name=name, bufs=8)

        def load_x(b):
            xT[b] = [
                hpool.tile([128, N], BF, tag=f"xT{i}", name=f"xT{i}", bufs=2)
                for i in range(6)
            ]
            eng = nc.scalar if b == 0 else nc.sync
            for i in range(6):
                eng.dma_start(xT[b][i][:], xT_d[b, 128 * i : 128 * (i + 1), :])

        def q_unit(b, m):
            # qkT[m] c-major [128, 1176] for q (m<6) / k (m>=6) rows
            if m == 0:
                qkT[b] = [None] * 12
            t = hpool.tile([128, N], BF, tag=f"qkT{m}", name=f"qkT{m}", bufs=2)
            qkT[b][m] = t
            for n0, nsz in NCHUNK:
                ps = psum("psQ")
                for kt in range(6):
                    nc.tensor.matmul(
                        ps[:, 0:nsz],
                        wq[kt][:, 128 * m : 128 * (m + 1)],
                        xT[b][kt][:, n0 : n0 + nsz],
                        start=(kt == 0),
                        stop=(kt == 5),
                    )
                # evac on scalar engine (gpsimd cannot read PSUM)
                nc.scalar.activation(
                    t[:, n0 : n0 + nsz],
                    ps[:, 0:nsz],
                    AF.Identity,
                    bias=bqkp[:, m : m + 1],
                )

        def v_unit(b, pt):
            # pos-major v_ext [psz, 12*65] with ones col at 64 of each head.
            # both c-chunks in one 2-bank claim -> ONE merged evac add
            p0, psz = POS_T[pt]
            if pt == 0:
                v_ext[b] = [None] * 10
            vt = hpool.tile([128, H * 65], BF, tag=f"vx{pt}", name=f"vx{pt}", bufs=2)
            v_ext[b][pt] = vt
            if b < 2:
                # two rotation slots; evac only writes the 64 v columns, so
                # ones persist across later batches
                nc.vector.memset(
                    vt[:].rearrange("p (h e) -> p h e", e=65)[:, :, 64:65], 1.0
                )
            for ci, (c0, csz) in enumerate(CCHUNK):
                ps = psum("psV")
                for kt in range(6):
                    nc.tensor.matmul(
                        ps[0:psz, 0:csz],
                        xT[b][kt][:, p0 : p0 + psz],
                        wv[kt][:, c0 : c0 + csz],
                        start=(kt == 0),
                        stop=(kt == 5),
                    )
                nh = csz // 64
                h0 = c0 // 64
                nc.vector.tensor_add(
                    vt[0:psz].rearrange("p (h e) -> p h e", e=65)[
                        :, h0 : h0 + nh, 0:64
                    ],
                    ps[0:psz, 0:csz].rearrange("p (h d) -> p h d", d=64),
                    vb_bc[0:psz, c0 : c0 + csz].rearrange("p (h d) -> p h d", d=64),
                )

        def pool_ct(b, ct):
            # sum 4x4 blocks of q_s -> agT (c-major). On VECTOR: gpsimd must
            # stay broadcast-only (lib swaps + in-order blocking starve the
            # norm-chain broadcasts otherwise)
            if ct == 0:
                agT[b] = []
            t1 = wpool.tile([128, 196], F32, tag="t1", bufs=1)
            qs = qkT[b][ct][:, N_MT:N]  # [128, 784], idx = i*28 + aj*4 + dj
            q4 = qs.rearrange("p (x dj) -> p x dj", dj=4)
            nc.vector.tensor_add(t1[:, 0:196], q4[:, :, 0:1], q4[:, :, 1:2])
            nc.vector.tensor_add(t1[:, 0:196], t1[:, 0:196], q4[:, :, 2:3])
            nc.vector.tensor_add(t1[:, 0:196], t1[:, 0:196], q4[:, :, 3:4])
            ag = hpool.tile([128, A], BF, tag=f"ag{ct}", name=f"ag{ct}", bufs=2)
            agT[b].append(ag)
            # t1 idx = 28*ai + 7*di + aj -> view (ai, aj, di)
            t4 = t1[:, 0:196].rearrange("p (ai di aj) -> p ai aj di", ai=7, di=4)
            t2 = wpool.tile([128, A], F32, tag="t2")
            nc.vector.tensor_add(t2[:, 0:A], t4[:, :, :, 0:1], t4[:, :, :, 1:2])
            nc.vector.tensor_add(t2[:, 0:A], t2[:, 0:A], t4[:, :, :, 2:3])
            nc.vector.tensor_add(ag[:, 0:A], t2[:, 0:A], t4[:, :, :, 3:4])

        def norm_pre(pv, c0):
            # recip of the psum sumexp row, broadcast to 64 partitions.
            # (custom-DVE recip reading PSUM directly returns garbage on HW;
            # stage the sumexp row through SBUF first.) Split from the mul so
            # the in-order vector queue never head-of-line blocks on gpsimd.
            se = wpool.tile([1, 392], F32, tag="se", bufs=2)
            nc.vector.tensor_copy(se[:, 0:392], pv[64:65, c0 : c0 + 392])
            rc = wpool.tile([1, 392], F32, tag="rc", bufs=2)
            nc.vector.reciprocal_approx_fast(out=rc[:, 0:392], in_=se[:, 0:392])
            bc = wpool.tile([64, 392], F32, tag="bc", bufs=6)
            nc.gpsimd.partition_broadcast(bc[:], rc[0:1, 0:392])
            return bc

        def norm_mul(pv, c0, bc, dst):
            nc.vector.tensor_mul(dst, pv[0:64, c0 : c0 + 392], bc[:])

        # ---- attention for one head pair, split into schedulable chunks ----
        def pair_scores_s1(b, p2, st):
            qt = p2
            # stage 1 scores first: [keys, queries] per head over 4 key chunks.
            # Claim order matches exp (= psum evacuation) order so the 8-bank
            # rotation never waits, and e1 (pv1's dep) is computed earliest.
            st["s1"] = []
            for hp in range(2):
                qo = 64 * hp
                chunks = []
                st["s1"].append(chunks)
                for k0, ksz in KEY1_T:
                    ps = psum("psS1")
                    chunks.append(ps)
                    nc.tensor.matmul(
                        ps[0:ksz, 0:392],
                        qkT[b][6 + qt][qo : qo + 64, k0 : k0 + ksz],
                        qkT[b][qt][qo : qo + 64, 0:N_MT],
                        start=True,
                        stop=True,
                    )
        def pair_scores_s23(b, p2, st):
            qt = p2
            # stage 2 scores: [49x2 packed, keys] over 3 chunks
            st["s2"] = []
            for n0, nsz in NCHUNK:
                ps = psum("psS2")
                st["s2"].append(ps)
                for hp in range(2):
                    qo = 64 * hp
                    nc.tensor.matmul(
                        ps[qo : qo + 49, 0:nsz],
                        agT[b][qt][qo : qo + 64, 0:A],
                        qkT[b][6 + qt][qo : qo + 64, n0 : n0 + nsz],
                        start=True,
                        stop=True,
                    )
            # stage 3 scores: [49x2 packed (agents), queries] over 2 chunks
            st["s3"] = []
            for cc in range(2):
                ps = psum("psS3")
                st["s3"].append(ps)
                for hp in range(2):
                    qo = 64 * hp
                    nc.tensor.matmul(
                        ps[qo : qo + 49, 0:392],
                        agT[b][qt][qo : qo + 64, 0:A],
                        qkT[b][qt][qo : qo + 64, N_MT + 392 * cc : N_MT + 392 * (cc + 1)],
                        start=True,
                        stop=True,
                    )
            # exps (scalar engine) in the same order as the score claims.
            # bufs sized for TWO pairs in flight (cross-pair pipeline)
            st["e1"] = []
            for hp in range(2):
                e1s = []
                st["e1"].append(e1s)
                for j, (k0, ksz) in enumerate(KEY1_T):
                    e1 = wpool.tile([128, 392], BF, tag="e1", name="e1", bufs=16)
                    e1s.append(e1)
                    nc.scalar.activation(
                        e1[0:ksz, 0:392],
                        st["s1"][hp][j][0:ksz, 0:392],
                        AF.Exp,
                        scale=SCALE1,
                    )
            e2 = wpool.tile([128, N], BF, tag="e2", bufs=2)
            st["e2"] = e2
            for j, (n0, nsz) in enumerate(NCHUNK):
                nc.scalar.activation(
                    e2[0:113, n0 : n0 + nsz],
                    st["s2"][j][0:113, 0:nsz],
                    AF.Exp,
                    scale=SCALE23,
                )
            st["e3"] = []
            for cc in range(2):
                e3 = wpool.tile([128, 392], BF, tag="e3", name="e3", bufs=4)
                st["e3"].append(e3)
                nc.scalar.activation(
                    e3[0:113, 0:392], st["s3"][cc][0:113, 0:392], AF.Exp, scale=SCALE23
                )

        def pair_scores(b, p2, st):
            pair_scores_s1(b, p2, st)
            pair_scores_s23(b, p2, st)

        def pair_pv1_mm(b, p2, st):
            st["pv1"] = []
            for hp in range(2):
                pv = psum("psPV1")
                st["pv1"].append(pv)
                for j, (k0, ksz) in enumerate(KEY1_T):
                    nc.tensor.matmul(
                        pv[0:65, 0:392],
                        v_ext[b][j][0:ksz, 65 * (2 * p2 + hp) : 65 * (2 * p2 + hp) + 65],
                        st["e1"][hp][j][0:ksz, 0:392],
                        start=(j == 0),
                        stop=(j == 3),
                    )

        def pair_pv1_norm_pre(b, p2, st):
            st["bc1"] = [norm_pre(st["pv1"][hp], 0) for hp in range(2)]

        def pair_pv1_norm_mul(b, p2, st):
            qt = p2
            for hp in range(2):
                qo = 64 * hp
                norm_mul(
                    st["pv1"][hp], 0, st["bc1"][hp],
                    aoT[b][qt][qo : qo + 64, 0:N_MT],
                )

        def pair_transp(b, p2, st):
            # [113, keys] -> [keys, 113] in 10 chunks, via identity matmul
            st["eT"] = []
            for half in range(2):
                trp = ppool.tile([128, 5 * TSP], BF, tag="P", name="psTr", bufs=8)
                for kk in range(5):
                    kt = 5 * half + kk
                    p0, psz = POS_T[kt]
                    nc.tensor.transpose(
                        trp[0:psz, TSP * kk : TSP * kk + 113],
                        st["e2"][0:113, p0 : p0 + psz],
                        ident[0:113, 0:113],
                    )
                eT = wpool.tile([128, 5 * TSP], BF, tag="e2T", bufs=4)
                st["eT"].append(eT)
                # evac on vector: scalar is exp-saturated in the pair slot
                nc.vector.tensor_copy(eT[:, 0 : 5 * TSP], trp[:, 0 : 5 * TSP])

        def pair_pv2(b, p2, st, do_memset=False):
            # both heads per matmul: lhsT = full transposed tile (garbage rows
            # 49:63 only pollute unused output rows), rhs = 129-wide v_ext
            # slice [v_h0 | ones | v_h1]; the ones col yields both heads'
            # sumexp at out col 64. 10 matmuls instead of 20.
            do_memset = do_memset or pv2_calls[0] < 2
            pv2_calls[0] += 1
            pv2 = psum("psPV2")
            for kt, (p0, psz) in enumerate(POS_T):
                eT = st["eT"][kt // 5]
                cof = TSP * (kt % 5)
                nc.tensor.matmul(
                    pv2[0:113, 0:129],
                    eT[0:psz, cof : cof + 113],
                    v_ext[b][kt][0:psz, 130 * p2 : 130 * p2 + 129],
                    start=(kt == 0),
                    stop=(kt == 9),
                )
            av = wpool.tile([128, 65], BF, tag="avx", bufs=2)
            st["av"] = av
            if do_memset:
                # two rotation slots; ts_muls only write cols 0:64, so the
                # ones column persists across later pairs
                nc.vector.memset(av[0:113, 64:65], 1.0)
            avr = wpool.tile([128, 1], F32, tag="avr", bufs=2)
            nc.vector.reciprocal(avr[0:113, 0:1], pv2[0:113, 64:65])
            nc.vector.tensor_scalar_mul(av[0:49, 0:64], pv2[0:49, 0:64], avr[0:49, 0:1])
            nc.vector.tensor_scalar_mul(
                av[64:113, 0:64], pv2[64:113, 65:129], avr[64:113, 0:1]
            )

        def pair_pv3_mm(b, p2, st):
            st["pv3"] = []
            for hp in range(2):
                for cc in range(2):
                    pv = psum("psPV3")
                    st["pv3"].append(pv)
                    nc.tensor.matmul(
                        pv[0:65, 0:392],
                        st["av"][64 * hp : 64 * hp + 49, 0:65],
                        st["e3"][cc][64 * hp : 64 * hp + 49, 0:392],
                        start=True,
                        stop=True,
                    )

        def pair_pv3_norm_pre(b, p2, st):
            st["bc3"] = [norm_pre(pv, 0) for pv in st["pv3"]]

        def pair_pv3_norm_mul(b, p2, st):
            qt = p2
            for i, pv in enumerate(st["pv3"]):
                hp, cc = divmod(i, 2)
                norm_mul(
                    pv,
                    0,
                    st["bc3"][i],
                    aoT[b][qt][64 * hp : 64 * hp + 64, N_MT + 392 * cc : N_MT + 392 * (cc + 1)],
                )

        def proj_unit(b, pt):
            p0, psz = POS_T[pt]
            ob = wpool.tile([128, C], BF, tag="osb")
            for c0, csz in CCHUNK:
                ps = psum("psPJ")
                for kt in range(6):
                    nc.tensor.matmul(
                        ps[0:psz, 0:csz],
                        aoT[b][kt][:, p0 : p0 + psz],
                        wp[kt][:, c0 : c0 + csz],
                        start=(kt == 0),
                        stop=(kt == 5),
                    )
                nc.vector.tensor_add(
                    ob[0:psz, c0 : c0 + csz], ps[0:psz, 0:csz], pb_bc[0:psz, c0 : c0 + csz]
                )
            nc.sync.dma_start(out_d[b, p0 : p0 + psz, :], ob[0:psz, :])

        def qk_pool_unit(b, m):
            q_unit(b, m)
            if m < 6:
                pool_ct(b, m)

        def qkv_units(b):
            units = []
            for m in range(12):
                units.append(lambda m=m: qk_pool_unit(b, m))
            for pt in range(10):
                units.append(lambda pt=pt: v_unit(b, pt))
            return units

        # ---- schedule ----
        # aoT[b] aliases qkT[b] q-tiles 0-5: by the time a pair's norms write
        # a tile, that pair's s1/s3 score matmuls (its only q readers) are
        # done -- the tile framework serializes the WAR. Saves 14KB/partition
        # and lets proj(b) units 2-9 defer into batch b+1's pair loop as PE
        # fillers (the last batch finally gets filler work).
        load_x(0)
        units0 = qkv_units(0)
        for u in units0[:12]:
            u()
        load_wvp()
        load_x(1)
        for u in units0[12:]:
            u()

        prev = None
        for b in range(NB):
            aoT[b] = qkT[b][0:6]
            # fill order matters: proj(b-1) reads qkT[b-1] (same buf parity
            # as qkv(b+1)'s q_unit writes), so proj units must drain first.
            fill = []
            if b > 0:
                fill += [lambda pt=pt, pb=b - 1: proj_unit(pb, pt) for pt in range(2, 10)]
            if b + 1 < NB:
                fill += list(qkv_units(b + 1))
            if b + 2 < NB:
                load_x(b + 2)
            fi = 0
            # last batch has few fillers (proj of b-1 only): ration them
            # across iterations instead of exhausting them in the first two
            ration = 2 if b == NB - 1 else 10**9
            it_taken = [0]

            def take(n):
                nonlocal fi
                for _ in range(n):
                    if fi < len(fill) and it_taken[0] < ration:
                        fill[fi]()
                        fi += 1
                        it_taken[0] += 1

            # cross-pair software pipeline: iteration runs scores(+exps) of
            # pair p while the pv-stages of pair p-1 consume exps computed a
            # full iteration (~17us) earlier -- the scalar exp chain leaves
            # the PE critical path entirely.
            for p2 in range(6):
                st = {}
                it_taken[0] = 0
                if prev is None:
                    pair_scores(b, p2, st)
                    take(2)
                elif prev[1] == 5:
                    # batch boundary: pv-stage of (b-1, 5) woven with
                    # scores(b, 0). No takes until its pv3 norms land --
                    # the proj(b-1) fillers read the s-region they write
                    # (taking one earlier would deadlock the PE queue).
                    pb_, pp_, pst = prev
                    pair_pv1_mm(pb_, pp_, pst)
                    pair_transp(pb_, pp_, pst)
                    pair_pv1_norm_pre(pb_, pp_, pst)
                    pair_scores(b, p2, st)
                    pair_pv2(pb_, pp_, pst)
                    pair_pv1_norm_mul(pb_, pp_, pst)
                    proj_unit(pb_, 0)
                    pair_pv3_mm(pb_, pp_, pst)
                    pair_pv3_norm_pre(pb_, pp_, pst)
                    pair_pv3_norm_mul(pb_, pp_, pst)
                    proj_unit(pb_, 1)
                    take(4)
                elif fi < len(fill) and b < NB - 1:
                    pb_, pp_, pst = prev
                    pair_scores(b, p2, st)
                    pair_pv1_mm(pb_, pp_, pst)
                    pair_transp(pb_, pp_, pst)
                    pair_pv1_norm_pre(pb_, pp_, pst)
                    take(1)
                    pair_pv2(pb_, pp_, pst)
                    pair_pv1_norm_mul(pb_, pp_, pst)
                    take(1)
                    pair_pv3_mm(pb_, pp_, pst)
                    pair_pv3_norm_pre(pb_, pp_, pst)
                    take(1)
                    pair_pv3_norm_mul(pb_, pp_, pst)
                    take(2)
                else:
                    # filler-starved (late b3): weave scores between pv2 and
                    # pv3 so the av vector chain hides under the score
                    # matmuls (same shape as the batch-boundary branch)
                    pb_, pp_, pst = prev
                    pair_pv1_mm(pb_, pp_, pst)
                    pair_transp(pb_, pp_, pst)
                    pair_pv1_norm_pre(pb_, pp_, pst)
                    pair_pv2(pb_, pp_, pst)
                    pair_pv1_norm_mul(pb_, pp_, pst)
                    pair_scores(b, p2, st)
                    pair_pv3_mm(pb_, pp_, pst)
                    pair_pv3_norm_pre(pb_, pp_, pst)
                    take(2)
                    pair_pv3_norm_mul(pb_, pp_, pst)
                prev = (b, p2, st)
            it_taken[0] = -(10**9)  # drain remaining fillers unrationed
            take(len(fill))

        # drain: pv-stage of the final pair + last batch's proj
        pb_, pp_, pst = prev
        pair_pv1_mm(pb_, pp_, pst)
        pair_transp(pb_, pp_, pst)
        pair_pv1_norm_pre(pb_, pp_, pst)
        pair_pv2(pb_, pp_, pst)
        pair_pv1_norm_mul(pb_, pp_, pst)
        proj_unit(pb_, 0)
        proj_unit(pb_, 1)
        pair_pv3_mm(pb_, pp_, pst)
        pair_pv3_norm_pre(pb_, pp_, pst)
        pair_pv3_norm_mul(pb_, pp_, pst)
        for pt in range(2, 10):
            proj_unit(pb_, pt)

    nc.compile()
    return nc


def tc_ctx(nc):
    from contextlib import contextmanager

    @contextmanager
    def ctx():
        with tile.TileContext(nc) as tc, nc.allow_low_precision(reason="attn bf16"):
            with (
                tc.tile_pool(name="const", bufs=1) as cpool,
                tc.tile_pool(name="work", bufs=2) as wpool,
                tc.tile_pool(name="hold", bufs=1) as hpool,
                tc.tile_pool(name="psum", bufs=8, space="PSUM") as ppool,
            ):
                yield tc, cpool, wpool, hpool, ppool

    return ctx()


_PROGRAM = None


def _get_program():
    global _PROGRAM
    if _PROGRAM is None:
        _PROGRAM = build_program()
    return _PROGRAM


def _prep_inputs(x, qkv_w, qkv_b, proj_w, proj_b):
    bf = ml_dtypes.bfloat16
    x = np.asarray(x, dtype=np.float32)
    xT = np.ascontiguousarray(x.transpose(0, 2, 1)).astype(bf)  # [B, C, N]
    wqkT = np.ascontiguousarray(np.asarray(qkv_w, dtype=np.float32).T).astype(bf)
    wpjT = np.ascontiguousarray(np.asarray(proj_w, dtype=np.float32).T).astype(bf)
    qb = np.asarray(qkv_b, dtype=np.float32)
    vbb = np.broadcast_to(qb[2 * 768 :].astype(bf), (128, 768)).copy()
    bqkp = np.ascontiguousarray(qb[: 2 * 768].reshape(12, 128).T).astype(np.float32)
    pbb = np.broadcast_to(
        np.asarray(proj_b, dtype=np.float32).astype(bf), (128, 768)
    ).copy()
    in_maps = []
    for c in range(N_CORES):
        in_maps.append(
            {
                "xT": np.ascontiguousarray(xT[c * NB : (c + 1) * NB]),
                "wqkT": wqkT,
                "wpjT": wpjT,
                "vbb": vbb,
                "bqkp": bqkp,
                "pbb": pbb,
            }
        )
    return in_maps


def kernel(x, qkv_w, qkv_b, proj_w, proj_b, t_h=14, t_w=14, s_h=28, s_w=28, **kw):
    nc = _get_program()
    in_maps = _prep_inputs(x, qkv_w, qkv_b, proj_w, proj_b)
    res = bass_utils.run_bass_kernel_spmd(nc, in_maps, core_ids=list(range(N_CORES)))
    out = np.concatenate([res.results[c]["out"] for c in range(N_CORES)], axis=0)
    return np.asarray(out, dtype=np.float32)


if __name__ == "__main__":
    build_program()
    print("program built OK")

